# revision 13
# baseline (speedup 1.0000x reference)
"""GAT 2-layer message-passing network on 8 TRN2 NeuronCores (Bass/Tile).

v4: self-loops handled directly (not as edge slots); dense shared slot
layout — per (superblock, chunk) segment, per-block runs of shared length
maxcnt[b,q] laid back-to-back, one pad-to-128 per segment (12-15% padding
vs 65% in v3). Tiles may span two adjacent dst blocks; two one-hot planes
(iota, iota+128) against a tile-relative dloc make the MM schedule
core-independent. Local node table hloc [NPCp, R1] feeds a_dst windows,
self-loop terms, and the L2 attention stats without core-dependent offsets.

Pipeline:
 - Phase A: htab (4 chunk tensors, 768B rows [h|asrc|adst]) from x@W1aug,
   batched 4 blocks/DMA; hloc for the core's own nodes from xTloc.
 - Phase B: per sb: gather src rows; per-edge a_dst via two-plane
   one-hot-transpose MMs; ex=exp(lrelu(asrc+adst)); msg in gather buffer;
   two-plane one-hot accumulation MMs + self-loop term; normalize, relu;
   h2aug = relu @ W2aug; h2 kept in SBUF (h2all) + h2loc DRAM.
 - AllGather h2loc -> h2tab; repack to 256B rows.
 - Phase C: gather h2 rows; same two-plane scheme; self-loop terms from
   h2all; batched log-softmax via exp+ln; single strided output DMA.
"""
import sys

if "/opt/trn_rl_repo" not in sys.path:
    sys.path.insert(0, "/opt/trn_rl_repo")

import math
import numpy as np
import ml_dtypes

import concourse.bass as bass
import concourse.bacc as bacc
import concourse.mybir as mybir
import concourse.tile as tile
from concourse import bass_utils

P = 128
NEG = 0.2
NCHUNK = 4
NQUEUE = 4
MAXT = 8                 # tiles per dma_gather call
SENT = 1000.0            # sentinel dloc (exact in bf16, never matches iota)

from concourse import tile_sem_assignment as _tsa  # noqa: E402

if not getattr(_tsa.TileClockTick, "_qaware_patched", False):
    _orig_assign_tick = _tsa.TileClockTick._assign_tick

    def _qaware_assign_tick(self, inst):
        if (isinstance(inst, _tsa.DMAInst)
                and inst.engine == mybir.EngineType.Pool):
            self.next_sw_dma_idx = getattr(inst, "queue_num", 0) or 0
        return _orig_assign_tick(self, inst)

    _tsa.TileClockTick._assign_tick = _qaware_assign_tick
    _tsa.TileClockTick._qaware_patched = True


def _wrap16(flat):
    """[n] -> [128, n//16] wrapped in 16 partitions, replicated x8."""
    w = flat.reshape(-1, 16).T
    return np.tile(w, (8, 1))


# ----------------------------------------------------------------------------
# host-side data prep
# ----------------------------------------------------------------------------

def prep(inputs, cfg):
    N, F, H, C, CLS, NC = cfg["N"], cfg["F"], cfg["H"], cfg["C"], cfg["CLS"], cfg["NC"]
    SBG = cfg.get("SBG", 4)
    x = np.asarray(inputs["x"], np.float32)
    ei = np.asarray(inputs["edge_index"])
    W1 = np.asarray(inputs["W1"], np.float32)
    as1 = np.asarray(inputs["att_src1"], np.float32)
    ad1 = np.asarray(inputs["att_dst1"], np.float32)
    b1 = np.asarray(inputs["b1"], np.float32)
    W2 = np.asarray(inputs["W2"], np.float32)
    as2 = np.asarray(inputs["att_src2"], np.float32)
    ad2 = np.asarray(inputs["att_dst2"], np.float32)
    b2 = np.asarray(inputs["b2"], np.float32)

    HC = H * C
    R1 = HC + 2 * H
    RG = 128 * math.ceil(R1 / 128)
    NPC = N // NC
    NB = math.ceil(NPC / P)
    NPCp = NB * P
    NT = (N + P - 1) // P
    Np = NT * P
    CHB = 25088
    assert NT == 782 and 3 * CHB < Np
    assert CHB < 32768 and NPCp < 32768

    # ---- weights / constants -------------------------------------------------
    W1r = W1.reshape(F, H, C)
    Wsrc = np.einsum("fhc,hc->fh", W1r, as1)
    Wdst = np.einsum("fhc,hc->fh", W1r, ad1)
    W1aug = np.concatenate([W1, Wsrc, Wdst], axis=1)          # [F, R1]
    Wsrc2 = W2 @ as2.reshape(CLS, 1)
    Wdst2 = W2 @ ad2.reshape(CLS, 1)
    W2aug = np.concatenate([W2, Wsrc2, Wdst2], axis=1)        # [HC, 4]

    bf16 = ml_dtypes.bfloat16
    xT = np.zeros((F, Np), dtype=bf16)
    xT[:, :N] = x.T.astype(bf16)
    b1rep = np.tile(b1[None, :], (P, 1)).astype(bf16)
    b2all = np.tile(b2[None, :], (P, NB)).astype(np.float32)
    ar = np.arange(P, dtype=np.float32)
    iota2 = np.tile(np.concatenate([ar, ar + P])[None, :], (P, 1)).astype(bf16)
    ident = np.eye(P, dtype=bf16)
    iotac2 = np.stack([ar, ar + P], axis=1)                   # [P, 2] f32

    # ---- edges (no self loops) ----------------------------------------------
    src_all = ei[0].astype(np.int64)
    dst_all = ei[1].astype(np.int64)
    order = np.argsort(dst_all, kind="stable")
    src_s = src_all[order]
    dst_s = dst_all[order]
    # secondary sort by chunk within equal dst not needed; we filter per chunk
    chunk_s = src_s // CHB

    cnts = np.zeros((NC, NB, NCHUNK), np.int64)
    for c in range(NC):
        for b in range(NB):
            base = c * NPC + b * P
            hi = min(base + P, (c + 1) * NPC)
            lo_i = np.searchsorted(dst_s, base)
            hi_i = np.searchsorted(dst_s, hi)
            ch = chunk_s[lo_i:hi_i]
            for q in range(NCHUNK):
                cnts[c, b, q] = (ch == q).sum()
    maxcnt = cnts.max(axis=0)                                 # [NB, NCHUNK]

    # ---- shared slot layout --------------------------------------------------
    sblocks = [list(range(i, min(i + SBG, NB))) for i in range(0, NB, SBG)]
    sb_meta = []
    tile_base = 0
    for blist in sblocks:
        segs = []                 # per q: (tile_base_global, segT)
        run_start = {}            # (b, q) -> slot offset within segment
        sb_base = tile_base
        tiles = []                # per rel tile: list of (bi, plane) covered
        accum = {b: [] for b in blist}
        for q in range(NCHUNK):
            L = 0
            for b in blist:
                run_start[(b, q)] = L
                L += int(maxcnt[b, q])
            segT = math.ceil(L / P)
            segs.append((tile_base, segT))
            for t in range(segT):
                lo, hi = t * P, (t + 1) * P
                cov = [b for b in blist
                       if maxcnt[b, q] > 0
                       and run_start[(b, q)] < hi
                       and run_start[(b, q)] + maxcnt[b, q] > lo]
                assert 1 <= len(cov) <= 2 and cov[-1] - cov[0] == len(cov) - 1
                rel = tile_base + t - sb_base
                gb1 = cov[0]
                tiles.append(dict(rel=rel, gb1=gb1,
                                  mms=[(b - blist[0], b - gb1) for b in cov]))
                for b in cov:
                    accum[b].append((rel, b - gb1))
            tile_base += segT
        sb_meta.append(dict(base=sb_base, S=tile_base - sb_base, segs=segs,
                            blocks=blist, tiles=tiles, accum=accum,
                            run_start=run_start))
    Tsum = tile_base

    # tile gb1 lookup: global tile index -> gb1 (for per-core dloc fill)
    gb1_of = np.zeros(Tsum, np.int64)
    for sb in sb_meta:
        for td in sb["tiles"]:
            gb1_of[sb["base"] + td["rel"]] = td["gb1"]

    # ---- per-core slot tables ------------------------------------------------
    NG2 = (NPCp // P) * 8                # h2 gather groups per core (782)
    ihsrc_w = np.zeros((NC, P, Tsum * 8), np.int16)
    ihsrc2_w = np.zeros((NC, P, Tsum * 8), np.int16)
    subv_a = np.zeros((NC, P, Tsum), bf16)
    dloc2d = np.zeros((NC, P, Tsum), bf16)
    dlocT_a = np.zeros((NC, 1, Tsum * P), bf16)
    for c in range(NC):
        ihsrc = np.zeros(Tsum * P, np.int16)
        ihsrc2 = np.zeros(Tsum * P, np.int16)
        subv = np.zeros(Tsum * P, np.float32)
        dloc = np.full(Tsum * P, SENT, np.float32)
        lo = np.searchsorted(dst_s, c * NPC)
        hi = np.searchsorted(dst_s, (c + 1) * NPC)
        cs, cd, cq = src_s[lo:hi], dst_s[lo:hi], chunk_s[lo:hi]
        for sb in sb_meta:
            for q in range(NCHUNK):
                tb, segT = sb["segs"][q]
                seg0 = tb * P
                for b in sb["blocks"]:
                    n = int(cnts[c, b, q])
                    if n == 0:
                        continue
                    base = c * NPC + b * P
                    top = min(base + P, (c + 1) * NPC)
                    s0 = np.searchsorted(cd, base)
                    s1 = np.searchsorted(cd, top)
                    m = cq[s0:s1] == q
                    es, ed = cs[s0:s1][m], cd[s0:s1][m]
                    assert len(es) == n
                    s = seg0 + sb["run_start"][(b, q)]
                    sl = np.arange(s, s + n)
                    ihsrc[sl] = (es - q * CHB).astype(np.int16)
                    ec, er = es // NPC, es % NPC
                    ihsrc2[sl] = (ec * NG2 + (er >> 4)).astype(np.int16)
                    subv[sl] = (er & 15).astype(np.float32)
                    dloc[sl] = (ed - c * NPC - gb1_of[sl // P] * P).astype(
                        np.float32)
        assert dloc[dloc != SENT].max(initial=0) < 256
        assert dloc[dloc != SENT].min(initial=0) >= 0
        ihsrc_w[c] = _wrap16(ihsrc)
        ihsrc2_w[c] = _wrap16(ihsrc2)
        subv_a[c] = subv.reshape(Tsum, P).T.astype(bf16)
        dloc2d[c] = dloc.reshape(Tsum, P).T.astype(bf16)
        dlocT_a[c, 0] = dloc.astype(bf16)

    shared = {
        "xT": xT, "W1aug": W1aug.astype(bf16), "W2aug": W2aug.astype(bf16),
        "b1rep": b1rep, "b2all": b2all, "iota2": iota2, "ident": ident,
        "iotac2": iotac2, "onesk": np.ones((1, P), bf16),
    }
    in_maps = []
    for c in range(NC):
        m = dict(shared)
        xl = np.zeros((F, NPCp), dtype=bf16)
        xl[:, :NPC] = xT[:, c * NPC:c * NPC + NPC]
        m["xTloc"] = xl
        m["ihsrc"] = ihsrc_w[c]
        m["ihsrc2"] = ihsrc2_w[c]
        m["subv"] = subv_a[c]
        m["dloc2d"] = dloc2d[c]
        m["dlocT"] = dlocT_a[c]
        in_maps.append(m)

    meta = dict(cfg, R1=R1, RG=RG, HC=HC, NPC=NPC, NPCp=NPCp, NB=NB, NT=NT,
                Np=Np, CHB=CHB, Tsum=Tsum, sb_meta=sb_meta, SBG=SBG)
    return in_maps, meta


# ----------------------------------------------------------------------------
# device program
# ----------------------------------------------------------------------------

def _sub(ap, elem_off, dims):
    return bass.AP(ap.tensor, ap.offset + elem_off, [ap.ap[0], *list(dims)])


def build(meta, nc=None):
    N, F, H, C, CLS = meta["N"], meta["F"], meta["H"], meta["C"], meta["CLS"]
    NC, R1, RG, HC = meta["NC"], meta["R1"], meta["RG"], meta["HC"]
    NPC, NPCp, NB, Np = meta["NPC"], meta["NPCp"], meta["NB"], meta["Np"]
    CHB, Tsum = meta["CHB"], meta["Tsum"]
    sb_meta = meta["sb_meta"]
    R2 = CLS + 2
    RL2 = 64
    RUSE = HC + H

    f32, bf16, i16 = mybir.dt.float32, mybir.dt.bfloat16, mybir.dt.int16

    if nc is None:
        nc = bacc.Bacc("TRN2", target_bir_lowering=False, debug=False,
                       num_devices=NC, num_swdge_queues=NQUEUE)

    qrr = [0]

    def gather_split(out_tile, rel, segT, elem, table, ix_tile):
        done = 0
        while done < segT:
            tt = min(MAXT, segT - done)
            r = rel + done
            nc.gpsimd.dma_gather(
                bass.AP(out_tile[:].tensor, out_tile[:].offset + r * elem,
                        [out_tile[:].ap[0], [elem, tt], [1, elem]]),
                table,
                ix_tile[:, r * 8:(r + tt) * 8],
                tt * P, tt * P, elem,
                queue_num=qrr[0] % NQUEUE,
            )
            qrr[0] += 1
            done += tt

    xT_d = nc.dram_tensor("xT", [F, Np], bf16, kind="ExternalInput")
    xTl_d = nc.dram_tensor("xTloc", [F, NPCp], bf16, kind="ExternalInput")
    W1aug_d = nc.dram_tensor("W1aug", [F, R1], bf16, kind="ExternalInput")
    W2aug_d = nc.dram_tensor("W2aug", [HC, R2], bf16, kind="ExternalInput")
    b1rep_d = nc.dram_tensor("b1rep", [P, HC], bf16, kind="ExternalInput")
    b2all_d = nc.dram_tensor("b2all", [P, NB * CLS], f32, kind="ExternalInput")
    iota2_d = nc.dram_tensor("iota2", [P, 2 * P], bf16, kind="ExternalInput")
    ident_d = nc.dram_tensor("ident", [P, P], bf16, kind="ExternalInput")
    ihsrc_d = nc.dram_tensor("ihsrc", [P, Tsum * 8], i16, kind="ExternalInput")
    ihsrc2_d = nc.dram_tensor("ihsrc2", [P, Tsum * 8], i16, kind="ExternalInput")
    subv_d = nc.dram_tensor("subv", [P, Tsum], bf16, kind="ExternalInput")
    dloc_d = nc.dram_tensor("dloc2d", [P, Tsum], bf16, kind="ExternalInput")
    dlocT_d = nc.dram_tensor("dlocT", [1, Tsum * P], bf16, kind="ExternalInput")
    iotac2_d = nc.dram_tensor("iotac2", [P, 2], f32, kind="ExternalInput")
    onesk_d = nc.dram_tensor("onesk", [1, P], bf16, kind="ExternalInput")
    out_d = nc.dram_tensor("out", [NPC, CLS], f32, kind="ExternalOutput")

    CH_ROWS = [CHB, CHB, CHB, Np - 3 * CHB]
    htabs = [nc.dram_tensor(f"htab{q}", [CH_ROWS[q], RG], bf16, kind="Internal")
             for q in range(NCHUNK)]
    hloc = nc.dram_tensor("hloc", [NPCp, R1], bf16, kind="Internal")
    h2loc = nc.dram_tensor("h2loc", [NPCp, R2], f32, kind="Internal")
    NG2 = (NPCp // P) * 8                 # 16-node gather groups per core
    NR2 = NG2 * 16                        # AllGather rows per core (12512)
    h2tabG = nc.dram_tensor("h2tabG", [NC * NR2, R2], f32, kind="Internal",
                            addr_space="Shared" if NC > 4 else "Local")

    FA = min(P, F)
    FB = F - FA
    NCK = (HC + P - 1) // P
    GRP = 8

    with tile.TileContext(nc) as tc:
        with tc.tile_pool(name="const", bufs=1) as cp:
            w1a = cp.tile([FA, R1], bf16)
            nc.sync.dma_start(out=w1a[:], in_=W1aug_d[0:FA, :])
            w1b = cp.tile([FB, R1], bf16)
            nc.sync.dma_start(out=w1b[:], in_=W1aug_d[FA:F, :])
            w2s = []
            for k in range(NCK):
                kk = min(P, HC - k * P)
                w2k = cp.tile([kk, R2], bf16, name=f"w2k{k}")
                nc.sync.dma_start(out=w2k[:], in_=W2aug_d[k * P:k * P + kk, :])
                w2s.append(w2k)
            b1s = cp.tile([P, HC], bf16)
            nc.sync.dma_start(out=b1s[:], in_=b1rep_d[:, :])
            b2a = cp.tile([P, NB * CLS], f32)
            nc.sync.dma_start(out=b2a[:], in_=b2all_d[:, :])
            iot2 = cp.tile([P, 2 * P], bf16)
            nc.sync.dma_start(out=iot2[:], in_=iota2_d[:, :])
            idn = cp.tile([P, P], bf16)
            nc.sync.dma_start(out=idn[:], in_=ident_d[:, :])
            dlc = cp.tile([P, Tsum], bf16)
            nc.sync.dma_start(out=dlc[:], in_=dloc_d[:, :])
            iotc2 = cp.tile([P, 2], f32)
            nc.sync.dma_start(out=iotc2[:], in_=iotac2_d[:, :])
            onek = cp.tile([1, P], bf16)
            nc.sync.dma_start(out=onek[:], in_=onesk_d[:, :])
            svc = cp.tile([P, Tsum], bf16)
            nc.sync.dma_start(out=svc[:], in_=subv_d[:, :])
            vall = cp.tile([P, NB * CLS], f32)
            h2all = cp.tile([P, NB * R2], f32)

            # ---------------- Phase A: feature tables ------------------------
            with tc.tile_pool(name="pa", bufs=3) as pa, \
                 tc.tile_pool(name="psa", bufs=4, space="PSUM") as psa:
                # local full rows first (phase B's pad MMs need them early)
                for g0 in range(0, NPCp // P, GRP):
                    glen = min(GRP, NPCp // P - g0)
                    w = glen * P
                    xa = pa.tile([FA, GRP * P], bf16, tag="xla")
                    nc.sync.dma_start(out=xa[:, :w],
                                      in_=xTl_d[0:FA, g0 * P:g0 * P + w])
                    xb = pa.tile([FB, GRP * P], bf16, tag="xlb")
                    nc.sync.dma_start(out=xb[:, :w],
                                      in_=xTl_d[FA:F, g0 * P:g0 * P + w])
                    ssb = pa.tile([P, GRP * R1], bf16, tag="ssb")
                    for k in range(glen):
                        ps = psa.tile([P, R1], f32, tag="pss")
                        nc.tensor.matmul(out=ps[:], lhsT=xa[:, k * P:(k + 1) * P],
                                         rhs=w1a[:], start=True, stop=False)
                        nc.tensor.matmul(out=ps[:], lhsT=xb[:, k * P:(k + 1) * P],
                                         rhs=w1b[:], start=False, stop=True)
                        nc.vector.tensor_copy(out=ssb[:, k * R1:(k + 1) * R1],
                                              in_=ps[:])
                    nc.sync.dma_start(
                        out=bass.AP(hloc, g0 * P * R1,
                                    [[R1, P], [P * R1, glen], [1, R1]]),
                        in_=ssb[:, :glen * R1].rearrange(
                            "p (g r) -> p g r", g=glen))
                for q in range(NCHUNK):
                    nbq = CH_ROWS[q] // P
                    for g0 in range(0, nbq, GRP):
                        glen = min(GRP, nbq - g0)
                        col0 = q * CHB + g0 * P
                        w = glen * P
                        xa = pa.tile([FA, GRP * P], bf16, tag="xa")
                        nc.sync.dma_start(out=xa[:, :w],
                                          in_=xT_d[0:FA, col0:col0 + w])
                        xb = pa.tile([FB, GRP * P], bf16, tag="xb")
                        nc.sync.dma_start(out=xb[:, :w],
                                          in_=xT_d[FA:F, col0:col0 + w])
                        hsb = pa.tile([P, GRP * R1], bf16, tag="hsb")
                        for k in range(glen):
                            ph = psa.tile([P, R1], f32, tag="ph")
                            nc.tensor.matmul(out=ph[:], lhsT=xa[:, k * P:(k + 1) * P],
                                             rhs=w1a[:], start=True, stop=False)
                            nc.tensor.matmul(out=ph[:], lhsT=xb[:, k * P:(k + 1) * P],
                                             rhs=w1b[:], start=False, stop=True)
                            nc.vector.tensor_copy(out=hsb[:, k * R1:(k + 1) * R1],
                                                  in_=ph[:])
                        nc.sync.dma_start(
                            out=bass.AP(htabs[q], g0 * P * RG,
                                        [[RG, P], [P * RG, glen], [1, R1]]),
                            in_=hsb[:, :glen * R1].rearrange(
                                "p (g r) -> p g r", g=glen))

            # ---------------- Phase B: L1 edge pass --------------------------
            with tc.tile_pool(name="pbg", bufs=2) as pbg, \
                 tc.tile_pool(name="pbb", bufs=2) as pbb, \
                 tc.tile_pool(name="psb", bufs=2, space="PSUM") as psb, \
                 tc.tile_pool(name="pst", bufs=1, space="PSUM") as pst, \
                 tc.tile_pool(name="psh", bufs=1, space="PSUM") as psh, \
                 tc.tile_pool(name="psk", bufs=2, space="PSUM") as psk, \
                 tc.tile_pool(name="psa2", bufs=2, space="PSUM") as psa2:
                for sb in sb_meta:
                    base, S = sb["base"], sb["S"]
                    nblk = len(sb["blocks"])
                    b0 = sb["blocks"][0]
                    g = pbg.tile([P, S * RG], bf16, tag="g")
                    ixs = pbg.tile([P, S * 8], i16, tag="ixs")
                    nc.sync.dma_start(out=ixs[:],
                                      in_=ihsrc_d[:, base * 8:(base + S) * 8])
                    for q in range(NCHUNK):
                        tb, segT = sb["segs"][q]
                        if segT == 0:
                            continue
                        gather_split(g, tb - base, segT, RG, htabs[q][:, :], ixs)
                    # local rows window [P, nblk*R1]: h, asrc, adst of own nodes
                    hbl = pbg.tile([P, 4 * R1], bf16, tag="hbl")
                    nc.sync.dma_start(
                        out=hbl[:, :nblk * R1],
                        in_=bass.AP(hloc, b0 * P * R1,
                                    [[R1, P], [P * R1, nblk], [1, R1]]))
                    # O_T planes: [d, slot] one-hots via PE broadcast + is_equal
                    dlT = pbg.tile([1, S * P], bf16, tag="dlT")
                    nc.sync.dma_start(out=dlT[:],
                                      in_=dlocT_d[0:1, base * P:(base + S) * P])
                    oTa = pbg.tile([P, S * P], bf16, tag="oTa", bufs=1)
                    oTb = pbg.tile([P, S * P], bf16, tag="oTb", bufs=1)
                    for st in range(0, S * P, 512):
                        w = min(512, S * P - st)
                        stp = psk.tile([P, 512], f32, tag="stp")
                        nc.tensor.matmul(out=stp[:, :w], lhsT=onek[:],
                                         rhs=dlT[0:1, st:st + w],
                                         start=True, stop=True)
                        nc.vector.tensor_tensor(
                            out=oTa[:, st:st + w],
                            in0=iotc2[:, 0:1].to_broadcast([P, w]),
                            in1=stp[:, :w],
                            op=mybir.AluOpType.is_equal)
                        nc.vector.tensor_tensor(
                            out=oTb[:, st:st + w],
                            in0=iotc2[:, 1:2].to_broadcast([P, w]),
                            in1=stp[:, :w],
                            op=mybir.AluOpType.is_equal)
                    # per-edge a_dst via plane MMs -> PSUM [P, S*H]
                    pad = psa2.tile([P, S * H], f32, tag="pad")
                    for td in sb["tiles"]:
                        rel = td["rel"]
                        nmm = len(td["mms"])
                        for mi, (bi, plane) in enumerate(td["mms"]):
                            oT = oTa if plane == 0 else oTb
                            nc.tensor.matmul(
                                out=pad[:, rel * H:(rel + 1) * H],
                                lhsT=oT[:, rel * P:(rel + 1) * P],
                                rhs=hbl[:, bi * R1 + HC + H:bi * R1 + HC + 2 * H],
                                start=(mi == 0), stop=(mi == nmm - 1),
                                skip_group_check=True)
                    # ex = exp(lrelu(asrc+adst)) for all slots  [P, S*H]
                    ex = pbb.tile([P, S * H], f32, tag="ex", bufs=1)
                    nc.vector.tensor_tensor(
                        out=ex[:].rearrange("p (t h) -> p t h", t=S),
                        in0=_sub(g[:], HC, [[RG, S], [1, H]]),
                        in1=pad[:].rearrange("p (t h) -> p t h", t=S),
                        op=mybir.AluOpType.add)
                    tmp = pbb.tile([P, S * H], f32, tag="tmp", bufs=1)
                    nc.vector.tensor_scalar_mul(out=tmp[:], in0=ex[:], scalar1=NEG)
                    nc.vector.tensor_tensor(out=ex[:], in0=ex[:], in1=tmp[:],
                                            op=mybir.AluOpType.max)
                    exb = pbb.tile([P, S * H], bf16, tag="exb", bufs=1)
                    nc.scalar.activation(out=exb[:], in_=ex[:],
                                         func=mybir.ActivationFunctionType.Exp)
                    # msg in-place: cols 0:HC *= ex ; cols HC:HC+H = ex
                    nc.vector.tensor_tensor(
                        out=_sub(g[:], 0, [[RG, S], [C, H], [1, C]]),
                        in0=_sub(g[:], 0, [[RG, S], [C, H], [1, C]]),
                        in1=_sub(exb[:], 0, [[H, S], [1, H], [0, C]]),
                        op=mybir.AluOpType.mult)
                    nc.vector.tensor_copy(
                        out=_sub(g[:], HC, [[RG, S], [1, H]]),
                        in_=exb[:].rearrange("p (t h) -> p t h", t=S))
                    # one-hot planes [P, S*P]
                    oha = pbb.tile([P, S * P], bf16, tag="oha", bufs=1)
                    nc.vector.tensor_tensor(
                        out=oha[:].rearrange("p (t q) -> p t q", t=S),
                        in0=_sub(dlc[:], base, [[1, S], [0, P]]),
                        in1=_sub(iot2[:], 0, [[0, S], [1, P]]),
                        op=mybir.AluOpType.is_equal)
                    ohb = pbb.tile([P, S * P], bf16, tag="ohb", bufs=1)
                    nc.vector.tensor_tensor(
                        out=ohb[:].rearrange("p (t q) -> p t q", t=S),
                        in0=_sub(dlc[:], base, [[1, S], [0, P]]),
                        in1=_sub(iot2[:], P, [[0, S], [1, P]]),
                        op=mybir.AluOpType.is_equal)
                    # self-loop stats for the sb's blocks  [P, nblk*H]
                    exs = pbb.tile([P, 4 * H], f32, tag="exs")
                    nc.vector.tensor_tensor(
                        out=exs[:, :nblk * H].rearrange("p (b h) -> p b h", b=nblk),
                        in0=_sub(hbl[:], HC, [[R1, nblk], [1, H]]),
                        in1=_sub(hbl[:], HC + H, [[R1, nblk], [1, H]]),
                        op=mybir.AluOpType.add)
                    tms = pbb.tile([P, 4 * H], f32, tag="tms")
                    nc.vector.tensor_scalar_mul(out=tms[:, :nblk * H],
                                                in0=exs[:, :nblk * H], scalar1=NEG)
                    nc.vector.tensor_tensor(out=exs[:, :nblk * H],
                                            in0=exs[:, :nblk * H],
                                            in1=tms[:, :nblk * H],
                                            op=mybir.AluOpType.max)
                    exsb = pbb.tile([P, 4 * H], bf16, tag="exsb")
                    nc.scalar.activation(out=exsb[:, :nblk * H],
                                         in_=exs[:, :nblk * H],
                                         func=mybir.ActivationFunctionType.Exp)
                    # per-block accumulation + normalize + L2 prep
                    h2w = pbb.tile([P, 8 * R2], f32, tag="h2w")
                    for bi, b in enumerate(sb["blocks"]):
                        mms = sb["accum"][b]
                        pso = psb.tile([P, RUSE], f32, tag="pso")
                        for mi, (rel, plane) in enumerate(mms):
                            oh = oha if plane == 0 else ohb
                            nc.tensor.matmul(
                                out=pso[:],
                                lhsT=oh[:, rel * P:(rel + 1) * P],
                                rhs=g[:, rel * RG:rel * RG + RUSE],
                                start=(mi == 0), stop=(mi == len(mms) - 1))
                        # self-loop message + denominator
                        tmb = pbb.tile([P, HC], bf16, tag="tmb")
                        nc.vector.tensor_tensor(
                            out=tmb[:].rearrange("p (h c) -> p h c", h=H),
                            in0=_sub(hbl[:], bi * R1, [[C, H], [1, C]]),
                            in1=_sub(exsb[:], bi * H, [[1, H], [0, C]]),
                            op=mybir.AluOpType.mult)
                        o1p = pbb.tile([P, HC], f32, tag="o1p")
                        nc.vector.tensor_tensor(out=o1p[:], in0=pso[:, 0:HC],
                                                in1=tmb[:],
                                                op=mybir.AluOpType.add)
                        den = pbb.tile([P, H], f32, tag="den")
                        nc.vector.tensor_tensor(
                            out=den[:], in0=pso[:, HC:HC + H],
                            in1=exsb[:, bi * H:(bi + 1) * H],
                            op=mybir.AluOpType.add)
                        rde = pbb.tile([P, H], f32, tag="rde")
                        nc.vector.reciprocal(out=rde[:], in_=den[:])
                        o1 = pbb.tile([P, HC], bf16, tag="o1")
                        for hh in range(H):
                            nc.vector.tensor_scalar_mul(
                                out=o1[:, hh * C:(hh + 1) * C],
                                in0=o1p[:, hh * C:(hh + 1) * C],
                                scalar1=rde[:, hh:hh + 1])
                        nc.vector.tensor_tensor(out=o1[:], in0=o1[:], in1=b1s[:],
                                                op=mybir.AluOpType.add)
                        nc.vector.tensor_scalar_max(out=o1[:], in0=o1[:],
                                                    scalar1=0.0)
                        ph2 = psh.tile([P, R2], f32, tag="ph2")
                        for k in range(NCK):
                            kk = min(P, HC - k * P)
                            ptr = pst.tile([P, P], bf16, tag="ptr")
                            nc.tensor.transpose(out=ptr[:kk, :],
                                                in_=o1[:, k * P:k * P + kk],
                                                identity=idn[:])
                            rT = pbb.tile([P, P], bf16, tag="rT")
                            nc.vector.tensor_copy(out=rT[:kk, :], in_=ptr[:kk, :])
                            nc.tensor.matmul(out=ph2[:], lhsT=rT[:kk, :],
                                             rhs=w2s[k][:kk, :],
                                             start=(k == 0), stop=(k == NCK - 1))
                        nc.vector.tensor_copy(out=h2w[:, bi * R2:(bi + 1) * R2],
                                              in_=ph2[:])
                    nc.vector.tensor_copy(
                        out=h2all[:, b0 * R2:(b0 + nblk) * R2],
                        in_=h2w[:, :nblk * R2])
                    nc.sync.dma_start(
                        out=bass.AP(h2loc, b0 * P * R2,
                                    [[R2, P], [P * R2, nblk], [1, R2]]),
                        in_=h2w[:, :nblk * R2].rearrange(
                            "p (g r) -> p g r", g=nblk))

            # ---------------- AllGather (padded per-core rows) ---------------
            nc.gpsimd.collective_compute(
                "AllGather", mybir.AluOpType.bypass,
                replica_groups=[list(range(NC))],
                ins=[h2loc[0:NR2, :]], outs=[h2tabG[:, :]])

            # ---------------- Phase C: L2 edge pass --------------------------
            with tc.tile_pool(name="pcg", bufs=2) as pcg, \
                 tc.tile_pool(name="pcb", bufs=2) as pcb, \
                 tc.tile_pool(name="psc", bufs=2, space="PSUM") as psc, \
                 tc.tile_pool(name="psk2", bufs=2, space="PSUM") as psk2, \
                 tc.tile_pool(name="psd2", bufs=2, space="PSUM") as psd2:
                for sb in sb_meta:
                    base, S = sb["base"], sb["S"]
                    nblk = len(sb["blocks"])
                    b0 = sb["blocks"][0]
                    g2 = pcg.tile([P, S * RL2], f32, tag="g2")
                    ixs = pcg.tile([P, S * 8], i16, tag="ixs2")
                    nc.sync.dma_start(out=ixs[:],
                                      in_=ihsrc2_d[:, base * 8:(base + S) * 8])
                    # one table: 16 packed nodes per 256B row of h2tabG
                    gather_split(g2, 0, S, RL2,
                                 bass.AP(h2tabG, 0, [[RL2, NC * NG2], [1, RL2]]),
                                 ixs)
                    # select each slot's 4 values out of its 16-node row
                    msel = pcg.tile([P, S * 16], bf16, tag="msel")
                    nc.vector.tensor_tensor(
                        out=msel[:].rearrange("p (t s) -> p t s", t=S),
                        in0=_sub(svc[:], base, [[1, S], [0, 16]]),
                        in1=_sub(iot2[:], 0, [[0, S], [1, 16]]),
                        op=mybir.AluOpType.is_equal)
                    tmps = pcg.tile([P, S * RL2], f32, tag="tmps")
                    nc.vector.tensor_tensor(
                        out=tmps[:].rearrange("p (t f s) -> p t f s", t=S, f=R2),
                        in0=_sub(g2[:], 0, [[RL2, S], [1, R2], [R2, 16]]),
                        in1=_sub(msel[:], 0, [[16, S], [0, R2], [1, 16]]),
                        op=mybir.AluOpType.mult)
                    h2m = pcg.tile([P, S * R2], f32, tag="h2m")
                    nc.vector.tensor_reduce(
                        out=h2m[:],
                        in_=tmps[:].rearrange("p (m s) -> p m s", m=S * R2),
                        axis=mybir.AxisListType.X,
                        op=mybir.AluOpType.add)
                    # a_dst2 window from resident h2all
                    adw2 = pcg.tile([P, 8], bf16, tag="adw2")
                    nc.vector.tensor_copy(
                        out=adw2[:, :nblk],
                        in_=_sub(h2all[:], b0 * R2 + CLS + 1, [[R2, nblk]]))
                    dlT = pcg.tile([1, S * P], bf16, tag="dlT2")
                    nc.sync.dma_start(out=dlT[:],
                                      in_=dlocT_d[0:1, base * P:(base + S) * P])
                    oTa = pcg.tile([P, S * P], bf16, tag="oT2a", bufs=1)
                    oTb = pcg.tile([P, S * P], bf16, tag="oT2b", bufs=1)
                    for st in range(0, S * P, 512):
                        w = min(512, S * P - st)
                        stp = psk2.tile([P, 512], f32, tag="stp2")
                        nc.tensor.matmul(out=stp[:, :w], lhsT=onek[:],
                                         rhs=dlT[0:1, st:st + w],
                                         start=True, stop=True)
                        nc.vector.tensor_tensor(
                            out=oTa[:, st:st + w],
                            in0=iotc2[:, 0:1].to_broadcast([P, w]),
                            in1=stp[:, :w],
                            op=mybir.AluOpType.is_equal)
                        nc.vector.tensor_tensor(
                            out=oTb[:, st:st + w],
                            in0=iotc2[:, 1:2].to_broadcast([P, w]),
                            in1=stp[:, :w],
                            op=mybir.AluOpType.is_equal)
                    pad2 = psd2.tile([P, S], f32, tag="pad2")
                    for td in sb["tiles"]:
                        rel = td["rel"]
                        nmm = len(td["mms"])
                        for mi, (bi, plane) in enumerate(td["mms"]):
                            oT = oTa if plane == 0 else oTb
                            nc.tensor.matmul(
                                out=pad2[:, rel:rel + 1],
                                lhsT=oT[:, rel * P:(rel + 1) * P],
                                rhs=adw2[:, bi:bi + 1],
                                start=(mi == 0), stop=(mi == nmm - 1),
                                skip_group_check=True)
                    ex2 = pcb.tile([P, S], f32, tag="ex2")
                    nc.vector.tensor_tensor(
                        out=ex2[:],
                        in0=_sub(h2m[:], CLS, [[R2, S]]),
                        in1=pad2[:],
                        op=mybir.AluOpType.add)
                    tm2 = pcb.tile([P, S], f32, tag="tm2")
                    nc.vector.tensor_scalar_mul(out=tm2[:], in0=ex2[:], scalar1=NEG)
                    nc.vector.tensor_tensor(out=ex2[:], in0=ex2[:], in1=tm2[:],
                                            op=mybir.AluOpType.max)
                    nc.scalar.activation(out=ex2[:], in_=ex2[:],
                                         func=mybir.ActivationFunctionType.Exp)
                    m2 = pcb.tile([P, S * 3], bf16, tag="m2")
                    nc.vector.tensor_copy(out=_sub(m2[:], CLS, [[3, S]]), in_=ex2[:])
                    nc.vector.tensor_tensor(
                        out=_sub(m2[:], 0, [[3, S], [1, CLS]]),
                        in0=_sub(h2m[:], 0, [[R2, S], [1, CLS]]),
                        in1=_sub(m2[:], CLS, [[3, S], [0, CLS]]),
                        op=mybir.AluOpType.mult)
                    oha = pcb.tile([P, S * P], bf16, tag="oh2a", bufs=1)
                    nc.vector.tensor_tensor(
                        out=oha[:].rearrange("p (t q) -> p t q", t=S),
                        in0=_sub(dlc[:], base, [[1, S], [0, P]]),
                        in1=_sub(iot2[:], 0, [[0, S], [1, P]]),
                        op=mybir.AluOpType.is_equal)
                    ohb = pcb.tile([P, S * P], bf16, tag="oh2b", bufs=1)
                    nc.vector.tensor_tensor(
                        out=ohb[:].rearrange("p (t q) -> p t q", t=S),
                        in0=_sub(dlc[:], base, [[1, S], [0, P]]),
                        in1=_sub(iot2[:], P, [[0, S], [1, P]]),
                        op=mybir.AluOpType.is_equal)
                    # self-loop L2 stats [P, nblk]
                    ex2s = pcb.tile([P, 8], f32, tag="ex2s")
                    nc.vector.tensor_tensor(
                        out=ex2s[:, :nblk],
                        in0=_sub(h2all[:], b0 * R2 + CLS, [[R2, nblk]]),
                        in1=_sub(h2all[:], b0 * R2 + CLS + 1, [[R2, nblk]]),
                        op=mybir.AluOpType.add)
                    tm2s = pcb.tile([P, 8], f32, tag="tm2s")
                    nc.vector.tensor_scalar_mul(out=tm2s[:, :nblk],
                                                in0=ex2s[:, :nblk], scalar1=NEG)
                    nc.vector.tensor_tensor(out=ex2s[:, :nblk], in0=ex2s[:, :nblk],
                                            in1=tm2s[:, :nblk],
                                            op=mybir.AluOpType.max)
                    nc.scalar.activation(out=ex2s[:, :nblk], in_=ex2s[:, :nblk],
                                         func=mybir.ActivationFunctionType.Exp)
                    for bi, b in enumerate(sb["blocks"]):
                        mms = sb["accum"][b]
                        ps2 = psc.tile([P, 3], f32, tag="ps2")
                        for mi, (rel, plane) in enumerate(mms):
                            oh = oha if plane == 0 else ohb
                            nc.tensor.matmul(
                                out=ps2[:],
                                lhsT=oh[:, rel * P:(rel + 1) * P],
                                rhs=m2[:, rel * 3:(rel + 1) * 3],
                                start=(mi == 0), stop=(mi == len(mms) - 1))
                        tmp2 = pcb.tile([P, CLS], f32, tag="tmp2")
                        nc.vector.tensor_tensor(
                            out=tmp2[:],
                            in0=_sub(h2all[:], b * R2, [[1, CLS]]),
                            in1=ex2s[:, bi:bi + 1].to_broadcast([P, CLS]),
                            op=mybir.AluOpType.mult)
                        v0 = pcb.tile([P, CLS], f32, tag="v0")
                        nc.vector.tensor_tensor(out=v0[:], in0=ps2[:, 0:CLS],
                                                in1=tmp2[:],
                                                op=mybir.AluOpType.add)
                        den2 = pcb.tile([P, 1], f32, tag="den2")
                        nc.vector.tensor_tensor(out=den2[:], in0=ps2[:, CLS:CLS + 1],
                                                in1=ex2s[:, bi:bi + 1],
                                                op=mybir.AluOpType.add)
                        rd2 = pcb.tile([P, 1], f32, tag="rd2")
                        nc.vector.reciprocal(out=rd2[:], in_=den2[:])
                        nc.vector.tensor_scalar_mul(
                            out=vall[:, b * CLS:(b + 1) * CLS],
                            in0=v0[:], scalar1=rd2[:, 0:1])
                # batched log-softmax: out[:,2b+i] = -ln(1+exp(v_other-v_i))
                nc.vector.tensor_tensor(out=vall[:], in0=vall[:], in1=b2a[:],
                                        op=mybir.AluOpType.add)
                vsw = cp.tile([P, NB * CLS], f32, name="vsw")
                nc.vector.tensor_copy(
                    out=_sub(vsw[:], 0, [[CLS, NB]]),
                    in_=_sub(vall[:], 1, [[CLS, NB]]))
                nc.vector.tensor_copy(
                    out=_sub(vsw[:], 1, [[CLS, NB]]),
                    in_=_sub(vall[:], 0, [[CLS, NB]]))
                nc.vector.tensor_tensor(out=vsw[:], in0=vsw[:], in1=vall[:],
                                        op=mybir.AluOpType.subtract)
                nc.scalar.activation(out=vsw[:], in_=vsw[:],
                                     func=mybir.ActivationFunctionType.Exp)
                nc.vector.tensor_scalar_add(out=vsw[:], in0=vsw[:], scalar1=1.0)
                nc.scalar.activation(out=vsw[:], in_=vsw[:],
                                     func=mybir.ActivationFunctionType.Ln)
                nc.vector.tensor_scalar_mul(out=vsw[:], in0=vsw[:], scalar1=-1.0)
                nfull = NPC // P
                nc.sync.dma_start(
                    out=bass.AP(out_d, 0, [[CLS, P], [P * CLS, nfull], [1, CLS]]),
                    in_=vsw[:, :nfull * CLS].rearrange(
                        "p (g r) -> p g r", g=nfull))
                rows = NPC - nfull * P
                if rows:
                    nc.sync.dma_start(
                        out=out_d[nfull * P:NPC, :],
                        in_=vsw[:rows, nfull * CLS:(nfull + 1) * CLS])
    nc.finalize()
    return nc


def install_ntff_hook(so_path="/opt/axon/libaxon_pjrt.so"):
    import types
    import ctypes
    import contextlib
    import antenv

    if getattr(antenv, "axon_hooks", None) is not None:
        return
    lib = ctypes.CDLL(so_path)
    if not hasattr(lib, "axon_start_nrt_profile"):
        return
    lib.axon_start_nrt_profile.argtypes = [ctypes.POINTER(ctypes.c_int64),
                                           ctypes.c_size_t]
    lib.axon_start_nrt_profile.restype = ctypes.c_int64
    lib.axon_stop_nrt_profile.argtypes = [ctypes.c_char_p]
    lib.axon_stop_nrt_profile.restype = ctypes.c_int64

    @contextlib.contextmanager
    def _hook(output_dir, device_ids):
        import jax
        jax.devices()
        if device_ids:
            ids = (ctypes.c_int64 * len(device_ids))(*device_ids)
            rc = lib.axon_start_nrt_profile(ids, len(device_ids))
        else:
            rc = lib.axon_start_nrt_profile(None, 0)
        if rc != 0:
            raise RuntimeError(f"axon_start_nrt_profile rc={rc}")
        try:
            yield
        finally:
            n = lib.axon_stop_nrt_profile(str(output_dir).encode())
            print(f"ntff profile: {n} file(s) written to {output_dir}")

    mod = types.ModuleType("antenv.axon_hooks")
    _reg = [_hook]
    mod.set_axon_ntff_profile_hook = lambda h: _reg.__setitem__(0, h)
    mod.get_axon_ntff_profile_hook = lambda: _reg[0]
    sys.modules["antenv.axon_hooks"] = mod
    antenv.axon_hooks = mod


def run(inputs, cfg, trace=False, **kwargs):
    if trace:
        install_ntff_hook()
    in_maps, meta = prep(inputs, cfg)
    nc = build(meta)
    res = bass_utils.run_bass_kernel_spmd(
        nc, in_maps, core_ids=list(range(cfg["NC"])), trace=trace, **kwargs)
    out = np.concatenate([res.results[c]["out"] for c in range(cfg["NC"])], axis=0)
    return out, res


_CFG = dict(N=100000, F=165, H=4, C=64, CLS=2, NC=8)


def kernel(**inputs):
    """Full (unsharded) inputs -> full [N, 2] float32 log-softmax output."""
    out, _ = run(inputs, _CFG, trace=False)
    return np.ascontiguousarray(out.astype(np.float32))


# revision 28
# speedup vs baseline: 1.2888x; 1.2888x over previous
"""GAT 2-layer message-passing network on 8 TRN2 NeuronCores (Bass/Tile).

v4: self-loops handled directly (not as edge slots); dense shared slot
layout — per (superblock, chunk) segment, per-block runs of shared length
maxcnt[b,q] laid back-to-back, one pad-to-128 per segment (12-15% padding
vs 65% in v3). Tiles may span two adjacent dst blocks; two one-hot planes
(iota, iota+128) against a tile-relative dloc make the MM schedule
core-independent. Local node table hloc [NPCp, R1] feeds a_dst windows,
self-loop terms, and the L2 attention stats without core-dependent offsets.

Pipeline:
 - Phase A: htab (4 chunk tensors, 768B rows [h|asrc|adst]) from x@W1aug,
   batched 4 blocks/DMA; hloc for the core's own nodes from xTloc.
 - Phase B: per sb: gather src rows; per-edge a_dst via two-plane
   one-hot-transpose MMs; ex=exp(lrelu(asrc+adst)); msg in gather buffer;
   two-plane one-hot accumulation MMs + self-loop term; normalize, relu;
   h2aug = relu @ W2aug; h2 kept in SBUF (h2all) + h2loc DRAM.
 - AllGather h2loc -> h2tab; repack to 256B rows.
 - Phase C: gather h2 rows; same two-plane scheme; self-loop terms from
   h2all; batched log-softmax via exp+ln; single strided output DMA.
"""
import sys

if "/opt/trn_rl_repo" not in sys.path:
    sys.path.insert(0, "/opt/trn_rl_repo")

import math
import numpy as np
import ml_dtypes

import concourse.bass as bass
import concourse.bacc as bacc
import concourse.mybir as mybir
import concourse.tile as tile
from concourse import bass_utils

P = 128
NEG = 0.2
NCHUNK = 4
NQUEUE = 4
MAXT = 8                 # tiles per dma_gather call
SENT = 1000.0            # sentinel dloc (exact in bf16, never matches iota)

from concourse import tile_sem_assignment as _tsa  # noqa: E402

if not getattr(_tsa.TileClockTick, "_qaware_patched", False):
    _orig_assign_tick = _tsa.TileClockTick._assign_tick

    def _qaware_assign_tick(self, inst):
        if (isinstance(inst, _tsa.DMAInst)
                and inst.engine == mybir.EngineType.Pool):
            self.next_sw_dma_idx = getattr(inst, "queue_num", 0) or 0
        return _orig_assign_tick(self, inst)

    _tsa.TileClockTick._assign_tick = _qaware_assign_tick
    _tsa.TileClockTick._qaware_patched = True


def _wrap16(flat):
    """[n] -> [128, n//16] wrapped in 16 partitions, replicated x8."""
    w = flat.reshape(-1, 16).T
    return np.tile(w, (8, 1))


# ----------------------------------------------------------------------------
# host-side data prep
# ----------------------------------------------------------------------------

def prep(inputs, cfg):
    N, F, H, C, CLS, NC = cfg["N"], cfg["F"], cfg["H"], cfg["C"], cfg["CLS"], cfg["NC"]
    SBG = cfg.get("SBG", 4)
    x = np.asarray(inputs["x"], np.float32)
    ei = np.asarray(inputs["edge_index"])
    W1 = np.asarray(inputs["W1"], np.float32)
    as1 = np.asarray(inputs["att_src1"], np.float32)
    ad1 = np.asarray(inputs["att_dst1"], np.float32)
    b1 = np.asarray(inputs["b1"], np.float32)
    W2 = np.asarray(inputs["W2"], np.float32)
    as2 = np.asarray(inputs["att_src2"], np.float32)
    ad2 = np.asarray(inputs["att_dst2"], np.float32)
    b2 = np.asarray(inputs["b2"], np.float32)

    HC = H * C
    R1 = HC + 2 * H
    RG = 128 * math.ceil(R1 / 128)
    NPC = N // NC
    NB = math.ceil(NPC / P)
    NPCp = NB * P
    NT = (N + P - 1) // P
    Np = NT * P
    CHB = 25088
    assert NT == 782 and 3 * CHB < Np
    assert CHB < 32768 and NPCp < 32768

    # ---- weights / constants -------------------------------------------------
    W1r = W1.reshape(F, H, C)
    Wsrc = np.einsum("fhc,hc->fh", W1r, as1)
    Wdst = np.einsum("fhc,hc->fh", W1r, ad1)
    W1aug = np.concatenate([W1, Wsrc, Wdst], axis=1)          # [F, R1]
    Wsrc2 = W2 @ as2.reshape(CLS, 1)
    Wdst2 = W2 @ ad2.reshape(CLS, 1)
    W2aug = np.concatenate([W2, Wsrc2, Wdst2], axis=1)        # [HC, 4]

    bf16 = ml_dtypes.bfloat16
    xT = np.zeros((F, Np), dtype=bf16)
    xT[:, :N] = x.T.astype(bf16)
    b1rep = np.tile(b1[None, :], (P, 1)).astype(bf16)
    b2all = np.tile(b2[None, :], (P, NB)).astype(np.float32)
    ar = np.arange(P, dtype=np.float32)
    iota2 = np.tile(np.concatenate([ar, ar + P])[None, :], (P, 1)).astype(bf16)
    iotak = np.tile(np.repeat(np.arange(16, dtype=np.float32), 4)[None, :],
                    (P, 1)).astype(bf16)                      # [P, 64]
    ident = np.eye(P, dtype=bf16)
    iotac2 = np.stack([ar, ar + P], axis=1)                   # [P, 2] f32

    # ---- edges (no self loops) ----------------------------------------------
    src_all = ei[0].astype(np.int64)
    dst_all = ei[1].astype(np.int64)
    order = np.argsort(dst_all, kind="stable")
    src_s = src_all[order]
    dst_s = dst_all[order]
    # secondary sort by chunk within equal dst not needed; we filter per chunk
    chunk_s = src_s // CHB

    cnts = np.zeros((NC, NB, NCHUNK), np.int64)
    for c in range(NC):
        for b in range(NB):
            base = c * NPC + b * P
            hi = min(base + P, (c + 1) * NPC)
            lo_i = np.searchsorted(dst_s, base)
            hi_i = np.searchsorted(dst_s, hi)
            ch = chunk_s[lo_i:hi_i]
            for q in range(NCHUNK):
                cnts[c, b, q] = (ch == q).sum()
    maxcnt = cnts.max(axis=0)                                 # [NB, NCHUNK]

    # ---- shared slot layout --------------------------------------------------
    sblocks = [list(range(i, min(i + SBG, NB))) for i in range(0, NB, SBG)]
    sb_meta = []
    tile_base = 0
    for blist in sblocks:
        segs = []                 # per q: (tile_base_global, segT)
        run_start = {}            # (b, q) -> slot offset within segment
        sb_base = tile_base
        tiles = []                # per rel tile: list of (bi, plane) covered
        accum = {b: [] for b in blist}
        for q in range(NCHUNK):
            L = 0
            for b in blist:
                run_start[(b, q)] = L
                L += int(maxcnt[b, q])
            segT = math.ceil(L / P)
            segs.append((tile_base, segT))
            for t in range(segT):
                lo, hi = t * P, (t + 1) * P
                cov = [b for b in blist
                       if maxcnt[b, q] > 0
                       and run_start[(b, q)] < hi
                       and run_start[(b, q)] + maxcnt[b, q] > lo]
                assert 1 <= len(cov) <= 2 and cov[-1] - cov[0] == len(cov) - 1
                rel = tile_base + t - sb_base
                gb1 = cov[0]
                tiles.append(dict(rel=rel, gb1=gb1,
                                  mms=[(b - blist[0], b - gb1) for b in cov]))
                for b in cov:
                    accum[b].append((rel, b - gb1))
            tile_base += segT
        sb_meta.append(dict(base=sb_base, S=tile_base - sb_base, segs=segs,
                            blocks=blist, tiles=tiles, accum=accum,
                            run_start=run_start))
    Tsum = tile_base

    # tile gb1 lookup: global tile index -> gb1 (for per-core dloc fill)
    gb1_of = np.zeros(Tsum, np.int64)
    for sb in sb_meta:
        for td in sb["tiles"]:
            gb1_of[sb["base"] + td["rel"]] = td["gb1"]

    # ---- per-core slot tables ------------------------------------------------
    NG2 = (NPCp // P) * 8                # h2 gather groups per core (782)
    ihsrc_w = np.zeros((NC, P, Tsum * 8), np.int16)
    ihsrc2_w = np.zeros((NC, P, Tsum * 8), np.int16)
    subv_a = np.zeros((NC, P, Tsum), bf16)
    dloc2d = np.zeros((NC, P, Tsum), bf16)
    dlocT_a = np.zeros((NC, 1, Tsum * P), bf16)
    for c in range(NC):
        ihsrc = np.zeros(Tsum * P, np.int16)
        ihsrc2 = np.zeros(Tsum * P, np.int16)
        subv = np.zeros(Tsum * P, np.float32)
        dloc = np.full(Tsum * P, SENT, np.float32)
        lo = np.searchsorted(dst_s, c * NPC)
        hi = np.searchsorted(dst_s, (c + 1) * NPC)
        cs, cd, cq = src_s[lo:hi], dst_s[lo:hi], chunk_s[lo:hi]
        for sb in sb_meta:
            for q in range(NCHUNK):
                tb, segT = sb["segs"][q]
                seg0 = tb * P
                for b in sb["blocks"]:
                    n = int(cnts[c, b, q])
                    if n == 0:
                        continue
                    base = c * NPC + b * P
                    top = min(base + P, (c + 1) * NPC)
                    s0 = np.searchsorted(cd, base)
                    s1 = np.searchsorted(cd, top)
                    m = cq[s0:s1] == q
                    es, ed = cs[s0:s1][m], cd[s0:s1][m]
                    assert len(es) == n
                    s = seg0 + sb["run_start"][(b, q)]
                    sl = np.arange(s, s + n)
                    ihsrc[sl] = (es - q * CHB).astype(np.int16)
                    ec, er = es // NPC, es % NPC
                    ihsrc2[sl] = (ec * NG2 + (er >> 4)).astype(np.int16)
                    subv[sl] = (er & 15).astype(np.float32)
                    dloc[sl] = (ed - c * NPC - gb1_of[sl // P] * P).astype(
                        np.float32)
        assert dloc[dloc != SENT].max(initial=0) < 256
        assert dloc[dloc != SENT].min(initial=0) >= 0
        ihsrc_w[c] = _wrap16(ihsrc)
        ihsrc2_w[c] = _wrap16(ihsrc2)
        subv_a[c] = subv.reshape(Tsum, P).T.astype(bf16)
        dloc2d[c] = dloc.reshape(Tsum, P).T.astype(bf16)
        dlocT_a[c, 0] = dloc.astype(bf16)

    shared = {
        "xT": xT, "W1aug": W1aug.astype(bf16), "W2aug": W2aug.astype(bf16),
        "b1rep": b1rep, "b2all": b2all, "iota2": iota2, "iotak": iotak,
        "ident": ident, "iotac2": iotac2, "onesk": np.ones((1, P), bf16),
    }
    in_maps = []
    for c in range(NC):
        m = dict(shared)
        xl = np.zeros((F, NPCp), dtype=bf16)
        xl[:, :NPC] = xT[:, c * NPC:c * NPC + NPC]
        m["xTloc"] = xl
        m["ihsrc"] = ihsrc_w[c]
        m["ihsrc2"] = ihsrc2_w[c]
        m["subv"] = subv_a[c]
        m["dloc2d"] = dloc2d[c]
        m["dlocT"] = dlocT_a[c]
        in_maps.append(m)

    meta = dict(cfg, R1=R1, RG=RG, HC=HC, NPC=NPC, NPCp=NPCp, NB=NB, NT=NT,
                Np=Np, CHB=CHB, Tsum=Tsum, sb_meta=sb_meta, SBG=SBG)
    return in_maps, meta


# ----------------------------------------------------------------------------
# device program
# ----------------------------------------------------------------------------

def _sub(ap, elem_off, dims):
    return bass.AP(ap.tensor, ap.offset + elem_off, [ap.ap[0], *list(dims)])


def build(meta, nc=None):
    N, F, H, C, CLS = meta["N"], meta["F"], meta["H"], meta["C"], meta["CLS"]
    NC, R1, RG, HC = meta["NC"], meta["R1"], meta["RG"], meta["HC"]
    NPC, NPCp, NB, Np = meta["NPC"], meta["NPCp"], meta["NB"], meta["Np"]
    CHB, Tsum = meta["CHB"], meta["Tsum"]
    sb_meta = meta["sb_meta"]
    R2 = CLS + 2
    RL2 = 64
    RUSE = HC + H

    f32, bf16, i16 = mybir.dt.float32, mybir.dt.bfloat16, mybir.dt.int16

    if nc is None:
        nc = bacc.Bacc("TRN2", target_bir_lowering=False, debug=False,
                       num_devices=NC, num_swdge_queues=NQUEUE)

    qrr = [0]

    def gather_split(out_tile, rel, segT, elem, table, ix_tile, ix_base=0):
        done = 0
        while done < segT:
            tt = min(MAXT, segT - done)
            r = rel + done
            ix0 = ix_base + r
            nc.gpsimd.dma_gather(
                bass.AP(out_tile[:].tensor, out_tile[:].offset + r * elem,
                        [out_tile[:].ap[0], [elem, tt], [1, elem]]),
                table,
                ix_tile[:, ix0 * 8:(ix0 + tt) * 8],
                tt * P, tt * P, elem,
                queue_num=qrr[0] % NQUEUE,
            )
            qrr[0] += 1
            done += tt

    xT_d = nc.dram_tensor("xT", [F, Np], bf16, kind="ExternalInput")
    xTl_d = nc.dram_tensor("xTloc", [F, NPCp], bf16, kind="ExternalInput")
    W1aug_d = nc.dram_tensor("W1aug", [F, R1], bf16, kind="ExternalInput")
    W2aug_d = nc.dram_tensor("W2aug", [HC, R2], bf16, kind="ExternalInput")
    b1rep_d = nc.dram_tensor("b1rep", [P, HC], bf16, kind="ExternalInput")
    b2all_d = nc.dram_tensor("b2all", [P, NB * CLS], f32, kind="ExternalInput")
    iota2_d = nc.dram_tensor("iota2", [P, 2 * P], bf16, kind="ExternalInput")
    iotak_d = nc.dram_tensor("iotak", [P, 64], bf16, kind="ExternalInput")
    ident_d = nc.dram_tensor("ident", [P, P], bf16, kind="ExternalInput")
    ihsrc_d = nc.dram_tensor("ihsrc", [P, Tsum * 8], i16, kind="ExternalInput")
    ihsrc2_d = nc.dram_tensor("ihsrc2", [P, Tsum * 8], i16, kind="ExternalInput")
    subv_d = nc.dram_tensor("subv", [P, Tsum], bf16, kind="ExternalInput")
    dloc_d = nc.dram_tensor("dloc2d", [P, Tsum], bf16, kind="ExternalInput")
    dlocT_d = nc.dram_tensor("dlocT", [1, Tsum * P], bf16, kind="ExternalInput")
    iotac2_d = nc.dram_tensor("iotac2", [P, 2], f32, kind="ExternalInput")
    onesk_d = nc.dram_tensor("onesk", [1, P], bf16, kind="ExternalInput")
    out_d = nc.dram_tensor("out", [NPC, CLS], f32, kind="ExternalOutput")

    CH_ROWS = [CHB, CHB, CHB, Np - 3 * CHB]
    htabs = [nc.dram_tensor(f"htab{q}", [CH_ROWS[q], RG], bf16, kind="Internal")
             for q in range(NCHUNK)]
    hloc = nc.dram_tensor("hloc", [NPCp, R1], bf16, kind="Internal")
    h2loc = nc.dram_tensor("h2loc", [NPCp, R2], f32, kind="Internal")
    NG2 = (NPCp // P) * 8                 # 16-node gather groups per core
    NR2 = NG2 * 16                        # AllGather rows per core (12512)
    h2tabG = nc.dram_tensor("h2tabG", [NC * NR2, R2], f32, kind="Internal",
                            addr_space="Shared" if NC > 4 else "Local")
    h2tabL = nc.dram_tensor("h2tabL", [NC * NR2, R2], f32, kind="Internal")

    FA = min(P, F)
    FB = F - FA
    NCK = (HC + P - 1) // P
    GRP = 8

    with tile.TileContext(nc) as tc:
        with tc.tile_pool(name="const", bufs=1) as cp:
            w1a = cp.tile([FA, R1], bf16)
            nc.sync.dma_start(out=w1a[:], in_=W1aug_d[0:FA, :])
            w1b = cp.tile([FB, R1], bf16)
            nc.sync.dma_start(out=w1b[:], in_=W1aug_d[FA:F, :])
            w2s = []
            for k in range(NCK):
                kk = min(P, HC - k * P)
                w2k = cp.tile([kk, R2], bf16, name=f"w2k{k}")
                nc.sync.dma_start(out=w2k[:], in_=W2aug_d[k * P:k * P + kk, :])
                w2s.append(w2k)
            b1s = cp.tile([P, HC], bf16)
            nc.sync.dma_start(out=b1s[:], in_=b1rep_d[:, :])
            b2a = cp.tile([P, NB * CLS], f32)
            nc.sync.dma_start(out=b2a[:], in_=b2all_d[:, :])
            iot2 = cp.tile([P, 2 * P], bf16)
            nc.sync.dma_start(out=iot2[:], in_=iota2_d[:, :])
            idn = cp.tile([P, P], bf16)
            nc.sync.dma_start(out=idn[:], in_=ident_d[:, :])
            dlc = cp.tile([P, Tsum], bf16)
            nc.sync.dma_start(out=dlc[:], in_=dloc_d[:, :])
            iotc2 = cp.tile([P, 2], f32)
            nc.sync.dma_start(out=iotc2[:], in_=iotac2_d[:, :])
            onek = cp.tile([1, P], bf16)
            nc.sync.dma_start(out=onek[:], in_=onesk_d[:, :])
            svc = cp.tile([P, Tsum], bf16)
            nc.sync.dma_start(out=svc[:], in_=subv_d[:, :])
            iotk = cp.tile([P, 64], bf16)
            nc.sync.dma_start(out=iotk[:], in_=iotak_d[:, :])
            ixall = cp.tile([P, Tsum * 8], i16)
            nc.sync.dma_start(out=ixall[:], in_=ihsrc_d[:, :])
            ixall2 = cp.tile([P, Tsum * 8], i16)
            nc.sync.dma_start(out=ixall2[:], in_=ihsrc2_d[:, :])
            vall = cp.tile([P, NB * CLS], f32)
            h2all = cp.tile([P, NB * R2], f32)

            # ---------------- Phase A: feature tables ------------------------
            with tc.tile_pool(name="pa", bufs=3) as pa, \
                 tc.tile_pool(name="psa", bufs=4, space="PSUM") as psa:
                def a_pass(src_d, ncols, out_fn, tagp):
                    for g0 in range(0, ncols // P, GRP):
                        glen = min(GRP, ncols // P - g0)
                        w = glen * P
                        xa = pa.tile([FA, GRP * P], bf16, tag=f"xa{tagp}")
                        nc.sync.dma_start(out=xa[:, :w],
                                          in_=src_d[0][0:FA,
                                                       src_d[1] + g0 * P:
                                                       src_d[1] + g0 * P + w])
                        xb = pa.tile([FB, GRP * P], bf16, tag=f"xb{tagp}")
                        nc.sync.dma_start(out=xb[:, :w],
                                          in_=src_d[0][FA:F,
                                                       src_d[1] + g0 * P:
                                                       src_d[1] + g0 * P + w])
                        hsb = pa.tile([P, GRP * R1], bf16, tag=f"hs{tagp}")
                        for k in range(glen):
                            ph = psa.tile([P, R1], f32, tag=f"ph{tagp}")
                            nc.tensor.matmul(out=ph[:], lhsT=xa[:, k * P:(k + 1) * P],
                                             rhs=w1a[:], start=True, stop=False)
                            nc.tensor.matmul(out=ph[:], lhsT=xb[:, k * P:(k + 1) * P],
                                             rhs=w1b[:], start=False, stop=True)
                            nc.vector.tensor_copy(out=hsb[:, k * R1:(k + 1) * R1],
                                                  in_=ph[:])
                        out_fn(g0, glen, hsb)

                def htab_writer(q):
                    def wr(g0, glen, hsb):
                        nc.sync.dma_start(
                            out=bass.AP(htabs[q], g0 * P * RG,
                                        [[RG, P], [P * RG, glen], [1, R1]]),
                            in_=hsb[:, :glen * R1].rearrange(
                                "p (g r) -> p g r", g=glen))
                    return wr

                def hloc_writer(g0, glen, hsb):
                    nc.sync.dma_start(
                        out=bass.AP(hloc, g0 * P * R1,
                                    [[R1, P], [P * R1, glen], [1, R1]]),
                        in_=hsb[:, :glen * R1].rearrange(
                            "p (g r) -> p g r", g=glen))

                a_pass((xT_d, 0), CH_ROWS[0], htab_writer(0), "g")
                a_pass((xTl_d, 0), NPCp, hloc_writer, "l")
                for q in range(1, NCHUNK):
                    a_pass((xT_d, q * CHB), CH_ROWS[q], htab_writer(q), "g")

            # ---------------- Phase B: L1 edge pass --------------------------
            with tc.tile_pool(name="pbg", bufs=2) as pbg, \
                 tc.tile_pool(name="pbb", bufs=2) as pbb, \
                 tc.tile_pool(name="psb", bufs=1, space="PSUM") as psb, \
                 tc.tile_pool(name="pst", bufs=1, space="PSUM") as pst, \
                 tc.tile_pool(name="psh", bufs=1, space="PSUM") as psh, \
                 tc.tile_pool(name="psk", bufs=1, space="PSUM") as psk, \
                 tc.tile_pool(name="psa2", bufs=1, space="PSUM") as psa2:
                for sb in sb_meta:
                    base, S = sb["base"], sb["S"]
                    nblk = len(sb["blocks"])
                    b0 = sb["blocks"][0]
                    g = pbg.tile([P, S * RG], bf16, tag="g")
                    for q in range(NCHUNK):
                        tb, segT = sb["segs"][q]
                        if segT == 0:
                            continue
                        gather_split(g, tb - base, segT, RG, htabs[q][:, :],
                                     ixall, ix_base=base)
                    # local rows window [P, nblk*R1]: h, asrc, adst of own nodes
                    hbl = pbg.tile([P, 4 * R1], bf16, tag="hbl")
                    nc.sync.dma_start(
                        out=hbl[:, :nblk * R1],
                        in_=bass.AP(hloc, b0 * P * R1,
                                    [[R1, P], [P * R1, nblk], [1, R1]]))
                    # O_T planes: [d, slot] one-hots via PE broadcast + is_equal
                    dlT = pbg.tile([1, S * P], bf16, tag="dlT")
                    nc.sync.dma_start(out=dlT[:],
                                      in_=dlocT_d[0:1, base * P:(base + S) * P])
                    oTa = pbg.tile([P, S * P], bf16, tag="oTa", bufs=1)
                    oTb = pbg.tile([P, S * P], bf16, tag="oTb", bufs=1)
                    for st in range(0, S * P, 512):
                        w = min(512, S * P - st)
                        stp = psk.tile([P, 512], f32, tag="stp")
                        nc.tensor.matmul(out=stp[:, :w], lhsT=onek[:],
                                         rhs=dlT[0:1, st:st + w],
                                         start=True, stop=True)
                        nc.vector.tensor_tensor(
                            out=oTa[:, st:st + w],
                            in0=iotc2[:, 0:1].to_broadcast([P, w]),
                            in1=stp[:, :w],
                            op=mybir.AluOpType.is_equal)
                        nc.vector.tensor_tensor(
                            out=oTb[:, st:st + w],
                            in0=iotc2[:, 1:2].to_broadcast([P, w]),
                            in1=stp[:, :w],
                            op=mybir.AluOpType.is_equal)
                    # per-edge a_dst via plane MMs -> PSUM [P, S*H]
                    pad = psa2.tile([P, S * H], f32, tag="pad")
                    for td in sb["tiles"]:
                        rel = td["rel"]
                        nmm = len(td["mms"])
                        for mi, (bi, plane) in enumerate(td["mms"]):
                            oT = oTa if plane == 0 else oTb
                            nc.tensor.matmul(
                                out=pad[:, rel * H:(rel + 1) * H],
                                lhsT=oT[:, rel * P:(rel + 1) * P],
                                rhs=hbl[:, bi * R1 + HC + H:bi * R1 + HC + 2 * H],
                                start=(mi == 0), stop=(mi == nmm - 1),
                                skip_group_check=True)
                    # ex = exp(lrelu(asrc+adst)) for all slots  [P, S*H]
                    ex = pbb.tile([P, S * H], f32, tag="ex", bufs=1)
                    nc.vector.tensor_tensor(
                        out=ex[:].rearrange("p (t h) -> p t h", t=S),
                        in0=_sub(g[:], HC, [[RG, S], [1, H]]),
                        in1=pad[:].rearrange("p (t h) -> p t h", t=S),
                        op=mybir.AluOpType.add)
                    nc.scalar.activation(out=ex[:], in_=ex[:],
                                         func=mybir.ActivationFunctionType.Prelu,
                                         alpha=NEG)
                    exb = pbb.tile([P, S * H], bf16, tag="exb", bufs=1)
                    nc.scalar.activation(out=exb[:], in_=ex[:],
                                         func=mybir.ActivationFunctionType.Exp)
                    # msg in-place: cols 0:HC *= ex ; cols HC:HC+H = ex
                    nc.vector.tensor_tensor(
                        out=_sub(g[:], 0, [[RG, S], [C, H], [1, C]]),
                        in0=_sub(g[:], 0, [[RG, S], [C, H], [1, C]]),
                        in1=_sub(exb[:], 0, [[H, S], [1, H], [0, C]]),
                        op=mybir.AluOpType.mult)
                    nc.vector.tensor_copy(
                        out=_sub(g[:], HC, [[RG, S], [1, H]]),
                        in_=exb[:].rearrange("p (t h) -> p t h", t=S))
                    # one-hot planes [P, S*P]
                    oha = pbb.tile([P, S * P], bf16, tag="oha", bufs=1)
                    nc.vector.tensor_tensor(
                        out=oha[:].rearrange("p (t q) -> p t q", t=S),
                        in0=_sub(dlc[:], base, [[1, S], [0, P]]),
                        in1=_sub(iot2[:], 0, [[0, S], [1, P]]),
                        op=mybir.AluOpType.is_equal)
                    ohb = pbb.tile([P, S * P], bf16, tag="ohb", bufs=1)
                    nc.vector.tensor_tensor(
                        out=ohb[:].rearrange("p (t q) -> p t q", t=S),
                        in0=_sub(dlc[:], base, [[1, S], [0, P]]),
                        in1=_sub(iot2[:], P, [[0, S], [1, P]]),
                        op=mybir.AluOpType.is_equal)
                    # self-loop stats for the sb's blocks  [P, nblk*H]
                    exs = pbb.tile([P, 4 * H], f32, tag="exs")
                    nc.vector.tensor_tensor(
                        out=exs[:, :nblk * H].rearrange("p (b h) -> p b h", b=nblk),
                        in0=_sub(hbl[:], HC, [[R1, nblk], [1, H]]),
                        in1=_sub(hbl[:], HC + H, [[R1, nblk], [1, H]]),
                        op=mybir.AluOpType.add)
                    nc.scalar.activation(out=exs[:, :nblk * H],
                                         in_=exs[:, :nblk * H],
                                         func=mybir.ActivationFunctionType.Prelu,
                                         alpha=NEG)
                    exsb = pbb.tile([P, 4 * H], bf16, tag="exsb")
                    nc.scalar.activation(out=exsb[:, :nblk * H],
                                         in_=exs[:, :nblk * H],
                                         func=mybir.ActivationFunctionType.Exp)
                    # per-block accumulation (one 4-bank PSUM tile, 512/blk)
                    pso4 = psb.tile([P, 4 * 512], f32, tag="pso4")
                    for bi, b in enumerate(sb["blocks"]):
                        mms = sb["accum"][b]
                        for mi, (rel, plane) in enumerate(mms):
                            oh = oha if plane == 0 else ohb
                            nc.tensor.matmul(
                                out=pso4[:, bi * 512:bi * 512 + RUSE],
                                lhsT=oh[:, rel * P:(rel + 1) * P],
                                rhs=g[:, rel * RG:rel * RG + RUSE],
                                start=(mi == 0), stop=(mi == len(mms) - 1))
                    # batched epilogue: self-loop msg, denominators, normalize
                    tmb = pbb.tile([P, 4 * HC], bf16, tag="tmb")
                    nc.vector.tensor_tensor(
                        out=tmb[:, :nblk * HC].rearrange(
                            "p (b h c) -> p b h c", b=nblk, h=H),
                        in0=_sub(hbl[:], 0, [[R1, nblk], [C, H], [1, C]]),
                        in1=_sub(exsb[:], 0, [[H, nblk], [1, H], [0, C]]),
                        op=mybir.AluOpType.mult)
                    o1p = pbb.tile([P, 4 * HC], f32, tag="o1p")
                    nc.vector.tensor_tensor(
                        out=o1p[:, :nblk * HC].rearrange(
                            "p (b c) -> p b c", b=nblk),
                        in0=_sub(pso4[:], 0, [[512, nblk], [1, HC]]),
                        in1=tmb[:, :nblk * HC].rearrange(
                            "p (b c) -> p b c", b=nblk),
                        op=mybir.AluOpType.add)
                    den = pbb.tile([P, 4 * H], f32, tag="den")
                    nc.vector.tensor_tensor(
                        out=den[:, :nblk * H].rearrange(
                            "p (b h) -> p b h", b=nblk),
                        in0=_sub(pso4[:], HC, [[512, nblk], [1, H]]),
                        in1=exsb[:, :nblk * H].rearrange(
                            "p (b h) -> p b h", b=nblk),
                        op=mybir.AluOpType.add)
                    rde = pbb.tile([P, 4 * H], f32, tag="rde")
                    nc.vector.reciprocal(out=rde[:, :nblk * H],
                                         in_=den[:, :nblk * H])
                    o1 = pbb.tile([P, 4 * HC], bf16, tag="o1")
                    nc.vector.tensor_tensor(
                        out=o1[:, :nblk * HC].rearrange(
                            "p (b h c) -> p b h c", b=nblk, h=H),
                        in0=o1p[:, :nblk * HC].rearrange(
                            "p (b h c) -> p b h c", b=nblk, h=H),
                        in1=_sub(rde[:], 0, [[H, nblk], [1, H], [0, C]]),
                        op=mybir.AluOpType.mult)
                    nc.vector.tensor_tensor(
                        out=o1[:, :nblk * HC].rearrange(
                            "p (b c) -> p b c", b=nblk),
                        in0=o1[:, :nblk * HC].rearrange(
                            "p (b c) -> p b c", b=nblk),
                        in1=_sub(b1s[:], 0, [[0, nblk], [1, HC]]),
                        op=mybir.AluOpType.add)
                    nc.vector.tensor_scalar_max(out=o1[:, :nblk * HC],
                                                in0=o1[:, :nblk * HC],
                                                scalar1=0.0)
                    # L2 features per block via PE transpose
                    h2w = pbb.tile([P, 8 * R2], f32, tag="h2w")
                    for bi, b in enumerate(sb["blocks"]):
                        ph2 = psh.tile([P, R2], f32, tag="ph2")
                        for k in range(NCK):
                            kk = min(P, HC - k * P)
                            ptr = pst.tile([P, P], bf16, tag="ptr")
                            nc.tensor.transpose(
                                out=ptr[:kk, :],
                                in_=o1[:, bi * HC + k * P:bi * HC + k * P + kk],
                                identity=idn[:])
                            rT = pbb.tile([P, P], bf16, tag="rT")
                            nc.vector.tensor_copy(out=rT[:kk, :], in_=ptr[:kk, :])
                            nc.tensor.matmul(out=ph2[:], lhsT=rT[:kk, :],
                                             rhs=w2s[k][:kk, :],
                                             start=(k == 0), stop=(k == NCK - 1))
                        nc.vector.tensor_copy(out=h2w[:, bi * R2:(bi + 1) * R2],
                                              in_=ph2[:])
                    nc.vector.tensor_copy(
                        out=h2all[:, b0 * R2:(b0 + nblk) * R2],
                        in_=h2w[:, :nblk * R2])
                    nc.sync.dma_start(
                        out=bass.AP(h2loc, b0 * P * R2,
                                    [[R2, P], [P * R2, nblk], [1, R2]]),
                        in_=h2w[:, :nblk * R2].rearrange(
                            "p (g r) -> p g r", g=nblk))

            # ---------------- AllGather (padded per-core rows) ---------------
            nc.gpsimd.collective_compute(
                "AllGather", mybir.AluOpType.bypass,
                replica_groups=[list(range(NC))],
                ins=[h2loc[0:NR2, :]], outs=[h2tabG[:, :]])
            # bounce to a local tensor: gathers from the Shared segment are slow
            nc.sync.dma_start(out=h2tabL[:, :], in_=h2tabG[:, :])

            # ---------------- Phase C: L2 edge pass --------------------------
            with tc.tile_pool(name="pcg", bufs=2) as pcg, \
                 tc.tile_pool(name="pcb", bufs=2) as pcb, \
                 tc.tile_pool(name="psc", bufs=1, space="PSUM") as psc, \
                 tc.tile_pool(name="psk2", bufs=2, space="PSUM") as psk2, \
                 tc.tile_pool(name="psd2", bufs=2, space="PSUM") as psd2:
                for sb in sb_meta:
                    base, S = sb["base"], sb["S"]
                    nblk = len(sb["blocks"])
                    b0 = sb["blocks"][0]
                    g2 = pcg.tile([P, S * RL2], f32, tag="g2")
                    # one table: 16 packed nodes per 256B row of h2tabL
                    gather_split(g2, 0, S, RL2,
                                 bass.AP(h2tabL, 0, [[RL2, NC * NG2], [1, RL2]]),
                                 ixall2, ix_base=base)
                    # select each slot's 4 values out of its 16-node row:
                    # expanded mask (k repeated R2x) + contiguous halving adds
                    msel = pcg.tile([P, S * RL2], bf16, tag="msel")
                    nc.vector.tensor_tensor(
                        out=msel[:].rearrange("p (t s) -> p t s", t=S),
                        in0=_sub(svc[:], base, [[1, S], [0, RL2]]),
                        in1=_sub(iotk[:], 0, [[0, S], [1, RL2]]),
                        op=mybir.AluOpType.is_equal)
                    tmps = pcg.tile([P, S * RL2], f32, tag="tmps")
                    nc.vector.tensor_tensor(
                        out=tmps[:], in0=g2[:], in1=msel[:],
                        op=mybir.AluOpType.mult)
                    for half in (32, 16, 8):
                        nc.vector.tensor_tensor(
                            out=_sub(tmps[:], 0, [[RL2, S], [1, half]]),
                            in0=_sub(tmps[:], 0, [[RL2, S], [1, half]]),
                            in1=_sub(tmps[:], half, [[RL2, S], [1, half]]),
                            op=mybir.AluOpType.add)
                    h2m = pcg.tile([P, S * R2], f32, tag="h2m")
                    nc.vector.tensor_tensor(
                        out=h2m[:].rearrange("p (t s) -> p t s", t=S),
                        in0=_sub(tmps[:], 0, [[RL2, S], [1, R2]]),
                        in1=_sub(tmps[:], R2, [[RL2, S], [1, R2]]),
                        op=mybir.AluOpType.add)
                    # a_dst2 window from resident h2all
                    adw2 = pcg.tile([P, 8], bf16, tag="adw2")
                    nc.vector.tensor_copy(
                        out=adw2[:, :nblk],
                        in_=_sub(h2all[:], b0 * R2 + CLS + 1, [[R2, nblk]]))
                    dlT = pcg.tile([1, S * P], bf16, tag="dlT2")
                    nc.sync.dma_start(out=dlT[:],
                                      in_=dlocT_d[0:1, base * P:(base + S) * P])
                    oTa = pcg.tile([P, S * P], bf16, tag="oT2a", bufs=1)
                    oTb = pcg.tile([P, S * P], bf16, tag="oT2b", bufs=1)
                    for st in range(0, S * P, 512):
                        w = min(512, S * P - st)
                        stp = psk2.tile([P, 512], f32, tag="stp2")
                        nc.tensor.matmul(out=stp[:, :w], lhsT=onek[:],
                                         rhs=dlT[0:1, st:st + w],
                                         start=True, stop=True)
                        nc.vector.tensor_tensor(
                            out=oTa[:, st:st + w],
                            in0=iotc2[:, 0:1].to_broadcast([P, w]),
                            in1=stp[:, :w],
                            op=mybir.AluOpType.is_equal)
                        nc.vector.tensor_tensor(
                            out=oTb[:, st:st + w],
                            in0=iotc2[:, 1:2].to_broadcast([P, w]),
                            in1=stp[:, :w],
                            op=mybir.AluOpType.is_equal)
                    pad2 = psd2.tile([P, S], f32, tag="pad2")
                    for td in sb["tiles"]:
                        rel = td["rel"]
                        nmm = len(td["mms"])
                        for mi, (bi, plane) in enumerate(td["mms"]):
                            oT = oTa if plane == 0 else oTb
                            nc.tensor.matmul(
                                out=pad2[:, rel:rel + 1],
                                lhsT=oT[:, rel * P:(rel + 1) * P],
                                rhs=adw2[:, bi:bi + 1],
                                start=(mi == 0), stop=(mi == nmm - 1),
                                skip_group_check=True)
                    ex2 = pcb.tile([P, S], f32, tag="ex2")
                    nc.vector.tensor_tensor(
                        out=ex2[:],
                        in0=_sub(h2m[:], CLS, [[R2, S]]),
                        in1=pad2[:],
                        op=mybir.AluOpType.add)
                    nc.scalar.activation(out=ex2[:], in_=ex2[:],
                                         func=mybir.ActivationFunctionType.Prelu,
                                         alpha=NEG)
                    nc.scalar.activation(out=ex2[:], in_=ex2[:],
                                         func=mybir.ActivationFunctionType.Exp)
                    m2 = pcb.tile([P, S * 3], bf16, tag="m2")
                    nc.vector.tensor_copy(out=_sub(m2[:], CLS, [[3, S]]), in_=ex2[:])
                    nc.vector.tensor_tensor(
                        out=_sub(m2[:], 0, [[3, S], [1, CLS]]),
                        in0=_sub(h2m[:], 0, [[R2, S], [1, CLS]]),
                        in1=_sub(m2[:], CLS, [[3, S], [0, CLS]]),
                        op=mybir.AluOpType.mult)
                    oha = pcb.tile([P, S * P], bf16, tag="oh2a", bufs=1)
                    nc.vector.tensor_tensor(
                        out=oha[:].rearrange("p (t q) -> p t q", t=S),
                        in0=_sub(dlc[:], base, [[1, S], [0, P]]),
                        in1=_sub(iot2[:], 0, [[0, S], [1, P]]),
                        op=mybir.AluOpType.is_equal)
                    ohb = pcb.tile([P, S * P], bf16, tag="oh2b", bufs=1)
                    nc.vector.tensor_tensor(
                        out=ohb[:].rearrange("p (t q) -> p t q", t=S),
                        in0=_sub(dlc[:], base, [[1, S], [0, P]]),
                        in1=_sub(iot2[:], P, [[0, S], [1, P]]),
                        op=mybir.AluOpType.is_equal)
                    # self-loop L2 stats [P, nblk]
                    ex2s = pcb.tile([P, 8], f32, tag="ex2s")
                    nc.vector.tensor_tensor(
                        out=ex2s[:, :nblk],
                        in0=_sub(h2all[:], b0 * R2 + CLS, [[R2, nblk]]),
                        in1=_sub(h2all[:], b0 * R2 + CLS + 1, [[R2, nblk]]),
                        op=mybir.AluOpType.add)
                    nc.scalar.activation(out=ex2s[:, :nblk], in_=ex2s[:, :nblk],
                                         func=mybir.ActivationFunctionType.Prelu,
                                         alpha=NEG)
                    nc.scalar.activation(out=ex2s[:, :nblk], in_=ex2s[:, :nblk],
                                         func=mybir.ActivationFunctionType.Exp)
                    # per-block accumulation into one shared PSUM bank
                    ps24 = psc.tile([P, 4 * P], f32, tag="ps24")
                    for bi, b in enumerate(sb["blocks"]):
                        mms = sb["accum"][b]
                        for mi, (rel, plane) in enumerate(mms):
                            oh = oha if plane == 0 else ohb
                            nc.tensor.matmul(
                                out=ps24[:, bi * P:bi * P + 3],
                                lhsT=oh[:, rel * P:(rel + 1) * P],
                                rhs=m2[:, rel * 3:(rel + 1) * 3],
                                start=(mi == 0), stop=(mi == len(mms) - 1))
                    # batched epilogue
                    tmp2 = pcb.tile([P, 8 * CLS], f32, tag="tmp2")
                    nc.vector.tensor_tensor(
                        out=tmp2[:, :nblk * CLS].rearrange(
                            "p (b c) -> p b c", b=nblk),
                        in0=_sub(h2all[:], b0 * R2, [[R2, nblk], [1, CLS]]),
                        in1=_sub(ex2s[:], 0, [[1, nblk], [0, CLS]]),
                        op=mybir.AluOpType.mult)
                    v0 = pcb.tile([P, 8 * CLS], f32, tag="v0")
                    nc.vector.tensor_tensor(
                        out=v0[:, :nblk * CLS].rearrange(
                            "p (b c) -> p b c", b=nblk),
                        in0=_sub(ps24[:], 0, [[P, nblk], [1, CLS]]),
                        in1=tmp2[:, :nblk * CLS].rearrange(
                            "p (b c) -> p b c", b=nblk),
                        op=mybir.AluOpType.add)
                    den2 = pcb.tile([P, 8], f32, tag="den2")
                    nc.vector.tensor_tensor(
                        out=den2[:, :nblk],
                        in0=_sub(ps24[:], CLS, [[P, nblk]]),
                        in1=ex2s[:, :nblk],
                        op=mybir.AluOpType.add)
                    rd2 = pcb.tile([P, 8], f32, tag="rd2")
                    nc.vector.reciprocal(out=rd2[:, :nblk], in_=den2[:, :nblk])
                    nc.vector.tensor_tensor(
                        out=vall[:, b0 * CLS:(b0 + nblk) * CLS].rearrange(
                            "p (b c) -> p b c", b=nblk),
                        in0=v0[:, :nblk * CLS].rearrange(
                            "p (b c) -> p b c", b=nblk),
                        in1=_sub(rd2[:], 0, [[1, nblk], [0, CLS]]),
                        op=mybir.AluOpType.mult)
                # batched log-softmax: out[:,2b+i] = -ln(1+exp(v_other-v_i))
                nc.vector.tensor_tensor(out=vall[:], in0=vall[:], in1=b2a[:],
                                        op=mybir.AluOpType.add)
                vsw = cp.tile([P, NB * CLS], f32, name="vsw")
                nc.vector.tensor_copy(
                    out=_sub(vsw[:], 0, [[CLS, NB]]),
                    in_=_sub(vall[:], 1, [[CLS, NB]]))
                nc.vector.tensor_copy(
                    out=_sub(vsw[:], 1, [[CLS, NB]]),
                    in_=_sub(vall[:], 0, [[CLS, NB]]))
                nc.vector.tensor_tensor(out=vsw[:], in0=vsw[:], in1=vall[:],
                                        op=mybir.AluOpType.subtract)
                nc.scalar.activation(out=vsw[:], in_=vsw[:],
                                     func=mybir.ActivationFunctionType.Exp)
                nc.vector.tensor_scalar_add(out=vsw[:], in0=vsw[:], scalar1=1.0)
                nc.scalar.activation(out=vsw[:], in_=vsw[:],
                                     func=mybir.ActivationFunctionType.Ln)
                nc.vector.tensor_scalar_mul(out=vsw[:], in0=vsw[:], scalar1=-1.0)
                nfull = NPC // P
                nc.sync.dma_start(
                    out=bass.AP(out_d, 0, [[CLS, P], [P * CLS, nfull], [1, CLS]]),
                    in_=vsw[:, :nfull * CLS].rearrange(
                        "p (g r) -> p g r", g=nfull))
                rows = NPC - nfull * P
                if rows:
                    nc.sync.dma_start(
                        out=out_d[nfull * P:NPC, :],
                        in_=vsw[:rows, nfull * CLS:(nfull + 1) * CLS])
    nc.finalize()
    return nc


def install_ntff_hook(so_path="/opt/axon/libaxon_pjrt.so"):
    import types
    import ctypes
    import contextlib
    import antenv

    if getattr(antenv, "axon_hooks", None) is not None:
        return
    lib = ctypes.CDLL(so_path)
    if not hasattr(lib, "axon_start_nrt_profile"):
        return
    lib.axon_start_nrt_profile.argtypes = [ctypes.POINTER(ctypes.c_int64),
                                           ctypes.c_size_t]
    lib.axon_start_nrt_profile.restype = ctypes.c_int64
    lib.axon_stop_nrt_profile.argtypes = [ctypes.c_char_p]
    lib.axon_stop_nrt_profile.restype = ctypes.c_int64

    @contextlib.contextmanager
    def _hook(output_dir, device_ids):
        import jax
        jax.devices()
        if device_ids:
            ids = (ctypes.c_int64 * len(device_ids))(*device_ids)
            rc = lib.axon_start_nrt_profile(ids, len(device_ids))
        else:
            rc = lib.axon_start_nrt_profile(None, 0)
        if rc != 0:
            raise RuntimeError(f"axon_start_nrt_profile rc={rc}")
        try:
            yield
        finally:
            n = lib.axon_stop_nrt_profile(str(output_dir).encode())
            print(f"ntff profile: {n} file(s) written to {output_dir}")

    mod = types.ModuleType("antenv.axon_hooks")
    _reg = [_hook]
    mod.set_axon_ntff_profile_hook = lambda h: _reg.__setitem__(0, h)
    mod.get_axon_ntff_profile_hook = lambda: _reg[0]
    sys.modules["antenv.axon_hooks"] = mod
    antenv.axon_hooks = mod


def run(inputs, cfg, trace=False, **kwargs):
    if trace:
        install_ntff_hook()
    in_maps, meta = prep(inputs, cfg)
    nc = build(meta)
    res = bass_utils.run_bass_kernel_spmd(
        nc, in_maps, core_ids=list(range(cfg["NC"])), trace=trace, **kwargs)
    out = np.concatenate([res.results[c]["out"] for c in range(cfg["NC"])], axis=0)
    return out, res


_CFG = dict(N=100000, F=165, H=4, C=64, CLS=2, NC=8)


def kernel(**inputs):
    """Full (unsharded) inputs -> full [N, 2] float32 log-softmax output."""
    out, _ = run(inputs, _CFG, trace=False)
    return np.ascontiguousarray(out.astype(np.float32))


# revision 36
# speedup vs baseline: 1.2934x; 1.0036x over previous
"""GAT 2-layer message-passing network on 8 TRN2 NeuronCores (Bass/Tile).

v4: self-loops handled directly (not as edge slots); dense shared slot
layout — per (superblock, chunk) segment, per-block runs of shared length
maxcnt[b,q] laid back-to-back, one pad-to-128 per segment (12-15% padding
vs 65% in v3). Tiles may span two adjacent dst blocks; two one-hot planes
(iota, iota+128) against a tile-relative dloc make the MM schedule
core-independent. Local node table hloc [NPCp, R1] feeds a_dst windows,
self-loop terms, and the L2 attention stats without core-dependent offsets.

Pipeline:
 - Phase A: htab (4 chunk tensors, 768B rows [h|asrc|adst]) from x@W1aug,
   batched 4 blocks/DMA; hloc for the core's own nodes from xTloc.
 - Phase B: per sb: gather src rows; per-edge a_dst via two-plane
   one-hot-transpose MMs; ex=exp(lrelu(asrc+adst)); msg in gather buffer;
   two-plane one-hot accumulation MMs + self-loop term; normalize, relu;
   h2aug = relu @ W2aug; h2 kept in SBUF (h2all) + h2loc DRAM.
 - AllGather h2loc -> h2tab; repack to 256B rows.
 - Phase C: gather h2 rows; same two-plane scheme; self-loop terms from
   h2all; batched log-softmax via exp+ln; single strided output DMA.
"""
import sys

if "/opt/trn_rl_repo" not in sys.path:
    sys.path.insert(0, "/opt/trn_rl_repo")

import math
import numpy as np
import ml_dtypes

import concourse.bass as bass
import concourse.bacc as bacc
import concourse.mybir as mybir
import concourse.tile as tile
from concourse import bass_utils

P = 128
NEG = 0.2
NCHUNK = 4
NQUEUE = 4
MAXT = 8                 # tiles per dma_gather call
SENT = 1000.0            # sentinel dloc (exact in bf16, never matches iota)

from concourse import tile_sem_assignment as _tsa  # noqa: E402

if not getattr(_tsa.TileClockTick, "_qaware_patched", False):
    _orig_assign_tick = _tsa.TileClockTick._assign_tick

    def _qaware_assign_tick(self, inst):
        if (isinstance(inst, _tsa.DMAInst)
                and inst.engine == mybir.EngineType.Pool):
            self.next_sw_dma_idx = getattr(inst, "queue_num", 0) or 0
        return _orig_assign_tick(self, inst)

    _tsa.TileClockTick._assign_tick = _qaware_assign_tick
    _tsa.TileClockTick._qaware_patched = True


def _wrap16(flat):
    """[n] -> [128, n//16] wrapped in 16 partitions, replicated x8."""
    w = flat.reshape(-1, 16).T
    return np.tile(w, (8, 1))


# ----------------------------------------------------------------------------
# host-side data prep
# ----------------------------------------------------------------------------

def prep(inputs, cfg):
    N, F, H, C, CLS, NC = cfg["N"], cfg["F"], cfg["H"], cfg["C"], cfg["CLS"], cfg["NC"]
    SBG = cfg.get("SBG", 4)
    x = np.asarray(inputs["x"], np.float32)
    ei = np.asarray(inputs["edge_index"])
    W1 = np.asarray(inputs["W1"], np.float32)
    as1 = np.asarray(inputs["att_src1"], np.float32)
    ad1 = np.asarray(inputs["att_dst1"], np.float32)
    b1 = np.asarray(inputs["b1"], np.float32)
    W2 = np.asarray(inputs["W2"], np.float32)
    as2 = np.asarray(inputs["att_src2"], np.float32)
    ad2 = np.asarray(inputs["att_dst2"], np.float32)
    b2 = np.asarray(inputs["b2"], np.float32)

    HC = H * C
    R1 = HC + 2 * H
    RG = 128 * math.ceil(R1 / 128)
    NPC = N // NC
    NB = math.ceil(NPC / P)
    NPCp = NB * P
    NT = (N + P - 1) // P
    Np = NT * P
    CHB = 25088
    assert NT == 782 and 3 * CHB < Np
    assert CHB < 32768 and NPCp < 32768

    # ---- weights / constants -------------------------------------------------
    W1r = W1.reshape(F, H, C)
    Wsrc = np.einsum("fhc,hc->fh", W1r, as1)
    Wdst = np.einsum("fhc,hc->fh", W1r, ad1)
    W1aug = np.concatenate([W1, Wsrc, Wdst], axis=1)          # [F, R1]
    Wsrc2 = W2 @ as2.reshape(CLS, 1)
    Wdst2 = W2 @ ad2.reshape(CLS, 1)
    W2aug = np.concatenate([W2, Wsrc2, Wdst2], axis=1)        # [HC, 4]

    bf16 = ml_dtypes.bfloat16
    xT = np.zeros((F, Np), dtype=bf16)
    xT[:, :N] = x.T.astype(bf16)
    b1rep = np.tile(b1[None, :], (P, 1)).astype(bf16)
    b2all = np.tile(b2[None, :], (P, NB)).astype(np.float32)
    ar = np.arange(P, dtype=np.float32)
    iota2 = np.tile(np.concatenate([ar, ar + P])[None, :], (P, 1)).astype(bf16)
    iotak = np.tile(np.repeat(np.arange(16, dtype=np.float32), 4)[None, :],
                    (P, 1)).astype(bf16)                      # [P, 64]
    ident = np.eye(P, dtype=bf16)
    iotac2 = np.stack([ar, ar + P], axis=1)                   # [P, 2] f32

    # ---- edges (no self loops) ----------------------------------------------
    src_all = ei[0].astype(np.int64)
    dst_all = ei[1].astype(np.int64)
    order = np.argsort(dst_all, kind="stable")
    src_s = src_all[order]
    dst_s = dst_all[order]
    # secondary sort by chunk within equal dst not needed; we filter per chunk
    chunk_s = src_s // CHB

    cnts = np.zeros((NC, NB, NCHUNK), np.int64)
    for c in range(NC):
        for b in range(NB):
            base = c * NPC + b * P
            hi = min(base + P, (c + 1) * NPC)
            lo_i = np.searchsorted(dst_s, base)
            hi_i = np.searchsorted(dst_s, hi)
            ch = chunk_s[lo_i:hi_i]
            for q in range(NCHUNK):
                cnts[c, b, q] = (ch == q).sum()
    maxcnt = cnts.max(axis=0)                                 # [NB, NCHUNK]

    # ---- shared slot layout --------------------------------------------------
    sblocks = [list(range(i, min(i + SBG, NB))) for i in range(0, NB, SBG)]
    sb_meta = []
    tile_base = 0
    for blist in sblocks:
        segs = []                 # per q: (tile_base_global, segT)
        run_start = {}            # (b, q) -> slot offset within segment
        sb_base = tile_base
        tiles = []                # per rel tile: list of (bi, plane) covered
        accum = {b: [] for b in blist}
        for q in range(NCHUNK):
            L = 0
            for b in blist:
                run_start[(b, q)] = L
                L += int(maxcnt[b, q])
            segT = math.ceil(L / P)
            segs.append((tile_base, segT))
            for t in range(segT):
                lo, hi = t * P, (t + 1) * P
                cov = [b for b in blist
                       if maxcnt[b, q] > 0
                       and run_start[(b, q)] < hi
                       and run_start[(b, q)] + maxcnt[b, q] > lo]
                assert 1 <= len(cov) <= 2 and cov[-1] - cov[0] == len(cov) - 1
                rel = tile_base + t - sb_base
                gb1 = cov[0]
                tiles.append(dict(rel=rel, gb1=gb1,
                                  mms=[(b - blist[0], b - gb1) for b in cov]))
                for b in cov:
                    accum[b].append((rel, b - gb1))
            tile_base += segT
        sb_meta.append(dict(base=sb_base, S=tile_base - sb_base, segs=segs,
                            blocks=blist, tiles=tiles, accum=accum,
                            run_start=run_start))
    Tsum = tile_base

    # tile gb1 lookup: global tile index -> gb1 (for per-core dloc fill)
    gb1_of = np.zeros(Tsum, np.int64)
    for sb in sb_meta:
        for td in sb["tiles"]:
            gb1_of[sb["base"] + td["rel"]] = td["gb1"]

    # ---- per-core slot tables ------------------------------------------------
    NG2 = (NPCp // P) * 8                # h2 gather groups per core (782)
    ihsrc_w = np.zeros((NC, P, Tsum * 8), np.int16)
    ihsrc2_w = np.zeros((NC, P, Tsum * 8), np.int16)
    subv_a = np.zeros((NC, P, Tsum), bf16)
    dloc2d = np.zeros((NC, P, Tsum), bf16)
    dlocT_a = np.zeros((NC, 1, Tsum * P), bf16)
    for c in range(NC):
        ihsrc = np.zeros(Tsum * P, np.int16)
        ihsrc2 = np.zeros(Tsum * P, np.int16)
        subv = np.zeros(Tsum * P, np.float32)
        dloc = np.full(Tsum * P, SENT, np.float32)
        lo = np.searchsorted(dst_s, c * NPC)
        hi = np.searchsorted(dst_s, (c + 1) * NPC)
        cs, cd, cq = src_s[lo:hi], dst_s[lo:hi], chunk_s[lo:hi]
        for sb in sb_meta:
            for q in range(NCHUNK):
                tb, segT = sb["segs"][q]
                seg0 = tb * P
                for b in sb["blocks"]:
                    n = int(cnts[c, b, q])
                    if n == 0:
                        continue
                    base = c * NPC + b * P
                    top = min(base + P, (c + 1) * NPC)
                    s0 = np.searchsorted(cd, base)
                    s1 = np.searchsorted(cd, top)
                    m = cq[s0:s1] == q
                    es, ed = cs[s0:s1][m], cd[s0:s1][m]
                    assert len(es) == n
                    s = seg0 + sb["run_start"][(b, q)]
                    sl = np.arange(s, s + n)
                    ihsrc[sl] = (es - q * CHB).astype(np.int16)
                    ec, er = es // NPC, es % NPC
                    ihsrc2[sl] = (ec * NG2 + (er >> 4)).astype(np.int16)
                    subv[sl] = (er & 15).astype(np.float32)
                    dloc[sl] = (ed - c * NPC - gb1_of[sl // P] * P).astype(
                        np.float32)
        assert dloc[dloc != SENT].max(initial=0) < 256
        assert dloc[dloc != SENT].min(initial=0) >= 0
        ihsrc_w[c] = _wrap16(ihsrc)
        ihsrc2_w[c] = _wrap16(ihsrc2)
        subv_a[c] = subv.reshape(Tsum, P).T.astype(bf16)
        dloc2d[c] = dloc.reshape(Tsum, P).T.astype(bf16)
        dlocT_a[c, 0] = dloc.astype(bf16)

    shared = {
        "xT": xT, "W1aug": W1aug.astype(bf16), "W2aug": W2aug.astype(bf16),
        "b1rep": b1rep, "b2all": b2all, "iota2": iota2, "iotak": iotak,
        "ident": ident, "iotac2": iotac2, "onesk": np.ones((1, P), bf16),
    }
    in_maps = []
    for c in range(NC):
        m = dict(shared)
        xl = np.zeros((F, NPCp), dtype=bf16)
        xl[:, :NPC] = xT[:, c * NPC:c * NPC + NPC]
        m["xTloc"] = xl
        m["ihsrc"] = ihsrc_w[c]
        m["ihsrc2"] = ihsrc2_w[c]
        m["subv"] = subv_a[c]
        m["dloc2d"] = dloc2d[c]
        m["dlocT"] = dlocT_a[c]
        in_maps.append(m)

    meta = dict(cfg, R1=R1, RG=RG, HC=HC, NPC=NPC, NPCp=NPCp, NB=NB, NT=NT,
                Np=Np, CHB=CHB, Tsum=Tsum, sb_meta=sb_meta, SBG=SBG)
    return in_maps, meta


# ----------------------------------------------------------------------------
# device program
# ----------------------------------------------------------------------------

def _sub(ap, elem_off, dims):
    return bass.AP(ap.tensor, ap.offset + elem_off, [ap.ap[0], *list(dims)])


def build(meta, nc=None):
    N, F, H, C, CLS = meta["N"], meta["F"], meta["H"], meta["C"], meta["CLS"]
    NC, R1, RG, HC = meta["NC"], meta["R1"], meta["RG"], meta["HC"]
    NPC, NPCp, NB, Np = meta["NPC"], meta["NPCp"], meta["NB"], meta["Np"]
    CHB, Tsum = meta["CHB"], meta["Tsum"]
    sb_meta = meta["sb_meta"]
    R2 = CLS + 2
    RL2 = 64
    RUSE = HC + H

    f32, bf16, i16 = mybir.dt.float32, mybir.dt.bfloat16, mybir.dt.int16

    if nc is None:
        nc = bacc.Bacc("TRN2", target_bir_lowering=False, debug=False,
                       num_devices=NC, num_swdge_queues=NQUEUE)

    qrr = [0]

    def gather_split(out_tile, rel, segT, elem, table, ix_tile, ix_base=0):
        done = 0
        while done < segT:
            tt = min(MAXT, segT - done)
            r = rel + done
            ix0 = ix_base + r
            nc.gpsimd.dma_gather(
                bass.AP(out_tile[:].tensor, out_tile[:].offset + r * elem,
                        [out_tile[:].ap[0], [elem, tt], [1, elem]]),
                table,
                ix_tile[:, ix0 * 8:(ix0 + tt) * 8],
                tt * P, tt * P, elem,
                queue_num=qrr[0] % NQUEUE,
            )
            qrr[0] += 1
            done += tt

    xT_d = nc.dram_tensor("xT", [F, Np], bf16, kind="ExternalInput")
    xTl_d = nc.dram_tensor("xTloc", [F, NPCp], bf16, kind="ExternalInput")
    W1aug_d = nc.dram_tensor("W1aug", [F, R1], bf16, kind="ExternalInput")
    W2aug_d = nc.dram_tensor("W2aug", [HC, R2], bf16, kind="ExternalInput")
    b1rep_d = nc.dram_tensor("b1rep", [P, HC], bf16, kind="ExternalInput")
    b2all_d = nc.dram_tensor("b2all", [P, NB * CLS], f32, kind="ExternalInput")
    iota2_d = nc.dram_tensor("iota2", [P, 2 * P], bf16, kind="ExternalInput")
    iotak_d = nc.dram_tensor("iotak", [P, 64], bf16, kind="ExternalInput")
    ident_d = nc.dram_tensor("ident", [P, P], bf16, kind="ExternalInput")
    ihsrc_d = nc.dram_tensor("ihsrc", [P, Tsum * 8], i16, kind="ExternalInput")
    ihsrc2_d = nc.dram_tensor("ihsrc2", [P, Tsum * 8], i16, kind="ExternalInput")
    subv_d = nc.dram_tensor("subv", [P, Tsum], bf16, kind="ExternalInput")
    dloc_d = nc.dram_tensor("dloc2d", [P, Tsum], bf16, kind="ExternalInput")
    dlocT_d = nc.dram_tensor("dlocT", [1, Tsum * P], bf16, kind="ExternalInput")
    iotac2_d = nc.dram_tensor("iotac2", [P, 2], f32, kind="ExternalInput")
    onesk_d = nc.dram_tensor("onesk", [1, P], bf16, kind="ExternalInput")
    out_d = nc.dram_tensor("out", [NPC, CLS], f32, kind="ExternalOutput")

    CH_ROWS = [CHB, CHB, CHB, Np - 3 * CHB]
    htabs = [nc.dram_tensor(f"htab{q}", [CH_ROWS[q], RG], bf16, kind="Internal")
             for q in range(NCHUNK)]
    hloc = nc.dram_tensor("hloc", [NPCp, R1], bf16, kind="Internal")
    h2loc = nc.dram_tensor("h2loc", [NPCp, R2], f32, kind="Internal")
    NG2 = (NPCp // P) * 8                 # 16-node gather groups per core
    NR2 = NG2 * 16                        # AllGather rows per core (12512)
    h2tabG = nc.dram_tensor("h2tabG", [NC * NR2, R2], f32, kind="Internal",
                            addr_space="Shared" if NC > 4 else "Local")
    h2tabL = nc.dram_tensor("h2tabL", [NC * NR2, R2], f32, kind="Internal")

    FA = min(P, F)
    FB = F - FA
    NCK = (HC + P - 1) // P
    GRP = 8

    with tile.TileContext(nc) as tc:
        with tc.tile_pool(name="const", bufs=1) as cp:
            w1a = cp.tile([FA, R1], bf16)
            nc.sync.dma_start(out=w1a[:], in_=W1aug_d[0:FA, :])
            w1b = cp.tile([FB, R1], bf16)
            nc.sync.dma_start(out=w1b[:], in_=W1aug_d[FA:F, :])
            w2s = []
            for k in range(NCK):
                kk = min(P, HC - k * P)
                w2k = cp.tile([kk, R2], bf16, name=f"w2k{k}")
                nc.sync.dma_start(out=w2k[:], in_=W2aug_d[k * P:k * P + kk, :])
                w2s.append(w2k)
            b1s = cp.tile([P, HC], bf16)
            nc.sync.dma_start(out=b1s[:], in_=b1rep_d[:, :])
            b2a = cp.tile([P, NB * CLS], f32)
            nc.sync.dma_start(out=b2a[:], in_=b2all_d[:, :])
            iot2 = cp.tile([P, 2 * P], bf16)
            nc.sync.dma_start(out=iot2[:], in_=iota2_d[:, :])
            idn = cp.tile([P, P], bf16)
            nc.sync.dma_start(out=idn[:], in_=ident_d[:, :])
            dlc = cp.tile([P, Tsum], bf16)
            nc.sync.dma_start(out=dlc[:], in_=dloc_d[:, :])
            iotc2 = cp.tile([P, 2], f32)
            nc.sync.dma_start(out=iotc2[:], in_=iotac2_d[:, :])
            onek = cp.tile([1, P], bf16)
            nc.sync.dma_start(out=onek[:], in_=onesk_d[:, :])
            svc = cp.tile([P, Tsum], bf16)
            nc.sync.dma_start(out=svc[:], in_=subv_d[:, :])
            iotk = cp.tile([P, 64], bf16)
            nc.sync.dma_start(out=iotk[:], in_=iotak_d[:, :])
            ixall = cp.tile([P, Tsum * 8], i16)
            nc.sync.dma_start(out=ixall[:], in_=ihsrc_d[:, :])
            vall = cp.tile([P, NB * CLS], f32)
            h2all = cp.tile([P, NB * R2], f32)

            # ---------------- Phase A: feature tables ------------------------
            with tc.tile_pool(name="pa", bufs=3) as pa, \
                 tc.tile_pool(name="psa", bufs=4, space="PSUM") as psa:
                def a_pass(src_d, ncols, out_fn, tagp):
                    for g0 in range(0, ncols // P, GRP):
                        glen = min(GRP, ncols // P - g0)
                        w = glen * P
                        xa = pa.tile([FA, GRP * P], bf16, tag=f"xa{tagp}")
                        nc.sync.dma_start(out=xa[:, :w],
                                          in_=src_d[0][0:FA,
                                                       src_d[1] + g0 * P:
                                                       src_d[1] + g0 * P + w])
                        xb = pa.tile([FB, GRP * P], bf16, tag=f"xb{tagp}")
                        nc.sync.dma_start(out=xb[:, :w],
                                          in_=src_d[0][FA:F,
                                                       src_d[1] + g0 * P:
                                                       src_d[1] + g0 * P + w])
                        hsb = pa.tile([P, GRP * R1], bf16, tag=f"hs{tagp}")
                        for k in range(glen):
                            ph = psa.tile([P, R1], f32, tag=f"ph{tagp}")
                            nc.tensor.matmul(out=ph[:], lhsT=xa[:, k * P:(k + 1) * P],
                                             rhs=w1a[:], start=True, stop=False)
                            nc.tensor.matmul(out=ph[:], lhsT=xb[:, k * P:(k + 1) * P],
                                             rhs=w1b[:], start=False, stop=True)
                            nc.vector.tensor_copy(out=hsb[:, k * R1:(k + 1) * R1],
                                                  in_=ph[:])
                        out_fn(g0, glen, hsb)

                def htab_writer(q):
                    def wr(g0, glen, hsb):
                        nc.sync.dma_start(
                            out=bass.AP(htabs[q], g0 * P * RG,
                                        [[RG, P], [P * RG, glen], [1, R1]]),
                            in_=hsb[:, :glen * R1].rearrange(
                                "p (g r) -> p g r", g=glen))
                    return wr

                def hloc_writer(g0, glen, hsb):
                    nc.sync.dma_start(
                        out=bass.AP(hloc, g0 * P * R1,
                                    [[R1, P], [P * R1, glen], [1, R1]]),
                        in_=hsb[:, :glen * R1].rearrange(
                            "p (g r) -> p g r", g=glen))

                a_pass((xT_d, 0), CH_ROWS[0], htab_writer(0), "g")
                a_pass((xTl_d, 0), NPCp, hloc_writer, "l")
                for q in range(1, NCHUNK):
                    a_pass((xT_d, q * CHB), CH_ROWS[q], htab_writer(q), "g")

            # ---------------- Phase B: L1 edge pass --------------------------
            with tc.tile_pool(name="pbg", bufs=2) as pbg, \
                 tc.tile_pool(name="pbb", bufs=2) as pbb, \
                 tc.tile_pool(name="psb", bufs=1, space="PSUM") as psb, \
                 tc.tile_pool(name="pst", bufs=1, space="PSUM") as pst, \
                 tc.tile_pool(name="psh", bufs=1, space="PSUM") as psh, \
                 tc.tile_pool(name="psk", bufs=1, space="PSUM") as psk, \
                 tc.tile_pool(name="psa2", bufs=1, space="PSUM") as psa2:
                for sb in sb_meta:
                    base, S = sb["base"], sb["S"]
                    nblk = len(sb["blocks"])
                    b0 = sb["blocks"][0]
                    g = pbg.tile([P, S * RG], bf16, tag="g")
                    for q in range(NCHUNK):
                        tb, segT = sb["segs"][q]
                        if segT == 0:
                            continue
                        gather_split(g, tb - base, segT, RG, htabs[q][:, :],
                                     ixall, ix_base=base)
                    # local rows window [P, nblk*R1]: h, asrc, adst of own nodes
                    hbl = pbg.tile([P, 4 * R1], bf16, tag="hbl")
                    nc.sync.dma_start(
                        out=hbl[:, :nblk * R1],
                        in_=bass.AP(hloc, b0 * P * R1,
                                    [[R1, P], [P * R1, nblk], [1, R1]]))
                    # O_T planes: [d, slot] one-hots via PE broadcast + is_equal
                    dlT = pbg.tile([1, S * P], bf16, tag="dlT")
                    nc.sync.dma_start(out=dlT[:],
                                      in_=dlocT_d[0:1, base * P:(base + S) * P])
                    oTa = pbg.tile([P, S * P], bf16, tag="oTa", bufs=1)
                    oTb = pbg.tile([P, S * P], bf16, tag="oTb", bufs=1)
                    for st in range(0, S * P, 512):
                        w = min(512, S * P - st)
                        stp = psk.tile([P, 512], f32, tag="stp")
                        nc.tensor.matmul(out=stp[:, :w], lhsT=onek[:],
                                         rhs=dlT[0:1, st:st + w],
                                         start=True, stop=True)
                        nc.vector.tensor_tensor(
                            out=oTa[:, st:st + w],
                            in0=iotc2[:, 0:1].to_broadcast([P, w]),
                            in1=stp[:, :w],
                            op=mybir.AluOpType.is_equal)
                        nc.vector.tensor_tensor(
                            out=oTb[:, st:st + w],
                            in0=iotc2[:, 1:2].to_broadcast([P, w]),
                            in1=stp[:, :w],
                            op=mybir.AluOpType.is_equal)
                    # per-edge a_dst via plane MMs -> PSUM [P, S*H]
                    pad = psa2.tile([P, S * H], f32, tag="pad")
                    for td in sb["tiles"]:
                        rel = td["rel"]
                        nmm = len(td["mms"])
                        for mi, (bi, plane) in enumerate(td["mms"]):
                            oT = oTa if plane == 0 else oTb
                            nc.tensor.matmul(
                                out=pad[:, rel * H:(rel + 1) * H],
                                lhsT=oT[:, rel * P:(rel + 1) * P],
                                rhs=hbl[:, bi * R1 + HC + H:bi * R1 + HC + 2 * H],
                                start=(mi == 0), stop=(mi == nmm - 1),
                                skip_group_check=True)
                    # ex = exp(lrelu(asrc+adst)) for all slots  [P, S*H]
                    ex = pbb.tile([P, S * H], f32, tag="ex")
                    nc.vector.tensor_tensor(
                        out=ex[:].rearrange("p (t h) -> p t h", t=S),
                        in0=_sub(g[:], HC, [[RG, S], [1, H]]),
                        in1=pad[:].rearrange("p (t h) -> p t h", t=S),
                        op=mybir.AluOpType.add)
                    nc.scalar.activation(out=ex[:], in_=ex[:],
                                         func=mybir.ActivationFunctionType.Prelu,
                                         alpha=NEG)
                    exb = pbb.tile([P, S * H], bf16, tag="exb")
                    nc.scalar.activation(out=exb[:], in_=ex[:],
                                         func=mybir.ActivationFunctionType.Exp)
                    # msg in-place: cols 0:HC *= ex ; cols HC:HC+H = ex
                    nc.vector.tensor_tensor(
                        out=_sub(g[:], 0, [[RG, S], [C, H], [1, C]]),
                        in0=_sub(g[:], 0, [[RG, S], [C, H], [1, C]]),
                        in1=_sub(exb[:], 0, [[H, S], [1, H], [0, C]]),
                        op=mybir.AluOpType.mult)
                    nc.vector.tensor_copy(
                        out=_sub(g[:], HC, [[RG, S], [1, H]]),
                        in_=exb[:].rearrange("p (t h) -> p t h", t=S))
                    # one-hot planes [P, S*P]
                    oha = pbb.tile([P, S * P], bf16, tag="oha")
                    nc.vector.tensor_tensor(
                        out=oha[:].rearrange("p (t q) -> p t q", t=S),
                        in0=_sub(dlc[:], base, [[1, S], [0, P]]),
                        in1=_sub(iot2[:], 0, [[0, S], [1, P]]),
                        op=mybir.AluOpType.is_equal)
                    ohb = pbb.tile([P, S * P], bf16, tag="ohb")
                    nc.vector.tensor_tensor(
                        out=ohb[:].rearrange("p (t q) -> p t q", t=S),
                        in0=_sub(dlc[:], base, [[1, S], [0, P]]),
                        in1=_sub(iot2[:], P, [[0, S], [1, P]]),
                        op=mybir.AluOpType.is_equal)
                    # self-loop stats for the sb's blocks  [P, nblk*H]
                    exs = pbb.tile([P, 4 * H], f32, tag="exs")
                    nc.vector.tensor_tensor(
                        out=exs[:, :nblk * H].rearrange("p (b h) -> p b h", b=nblk),
                        in0=_sub(hbl[:], HC, [[R1, nblk], [1, H]]),
                        in1=_sub(hbl[:], HC + H, [[R1, nblk], [1, H]]),
                        op=mybir.AluOpType.add)
                    nc.scalar.activation(out=exs[:, :nblk * H],
                                         in_=exs[:, :nblk * H],
                                         func=mybir.ActivationFunctionType.Prelu,
                                         alpha=NEG)
                    exsb = pbb.tile([P, 4 * H], bf16, tag="exsb")
                    nc.scalar.activation(out=exsb[:, :nblk * H],
                                         in_=exs[:, :nblk * H],
                                         func=mybir.ActivationFunctionType.Exp)
                    # per-block accumulation (one 4-bank PSUM tile, 512/blk)
                    pso4 = psb.tile([P, 4 * 512], f32, tag="pso4")
                    for bi, b in enumerate(sb["blocks"]):
                        mms = sb["accum"][b]
                        for mi, (rel, plane) in enumerate(mms):
                            oh = oha if plane == 0 else ohb
                            nc.tensor.matmul(
                                out=pso4[:, bi * 512:bi * 512 + RUSE],
                                lhsT=oh[:, rel * P:(rel + 1) * P],
                                rhs=g[:, rel * RG:rel * RG + RUSE],
                                start=(mi == 0), stop=(mi == len(mms) - 1))
                    # batched epilogue: self-loop msg, denominators, normalize
                    tmb = pbb.tile([P, 4 * HC], bf16, tag="tmb")
                    nc.vector.tensor_tensor(
                        out=tmb[:, :nblk * HC].rearrange(
                            "p (b h c) -> p b h c", b=nblk, h=H),
                        in0=_sub(hbl[:], 0, [[R1, nblk], [C, H], [1, C]]),
                        in1=_sub(exsb[:], 0, [[H, nblk], [1, H], [0, C]]),
                        op=mybir.AluOpType.mult)
                    o1p = pbb.tile([P, 4 * HC], f32, tag="o1p")
                    nc.vector.tensor_tensor(
                        out=o1p[:, :nblk * HC].rearrange(
                            "p (b c) -> p b c", b=nblk),
                        in0=_sub(pso4[:], 0, [[512, nblk], [1, HC]]),
                        in1=tmb[:, :nblk * HC].rearrange(
                            "p (b c) -> p b c", b=nblk),
                        op=mybir.AluOpType.add)
                    den = pbb.tile([P, 4 * H], f32, tag="den")
                    nc.vector.tensor_tensor(
                        out=den[:, :nblk * H].rearrange(
                            "p (b h) -> p b h", b=nblk),
                        in0=_sub(pso4[:], HC, [[512, nblk], [1, H]]),
                        in1=exsb[:, :nblk * H].rearrange(
                            "p (b h) -> p b h", b=nblk),
                        op=mybir.AluOpType.add)
                    rde = pbb.tile([P, 4 * H], f32, tag="rde")
                    nc.vector.reciprocal(out=rde[:, :nblk * H],
                                         in_=den[:, :nblk * H])
                    o1 = pbb.tile([P, 4 * HC], bf16, tag="o1")
                    nc.vector.tensor_tensor(
                        out=o1[:, :nblk * HC].rearrange(
                            "p (b h c) -> p b h c", b=nblk, h=H),
                        in0=o1p[:, :nblk * HC].rearrange(
                            "p (b h c) -> p b h c", b=nblk, h=H),
                        in1=_sub(rde[:], 0, [[H, nblk], [1, H], [0, C]]),
                        op=mybir.AluOpType.mult)
                    nc.vector.tensor_tensor(
                        out=o1[:, :nblk * HC].rearrange(
                            "p (b c) -> p b c", b=nblk),
                        in0=o1[:, :nblk * HC].rearrange(
                            "p (b c) -> p b c", b=nblk),
                        in1=_sub(b1s[:], 0, [[0, nblk], [1, HC]]),
                        op=mybir.AluOpType.add)
                    nc.vector.tensor_scalar_max(out=o1[:, :nblk * HC],
                                                in0=o1[:, :nblk * HC],
                                                scalar1=0.0)
                    # L2 features per block via PE transpose
                    h2w = pbb.tile([P, 8 * R2], f32, tag="h2w")
                    for bi, b in enumerate(sb["blocks"]):
                        ph2 = psh.tile([P, R2], f32, tag="ph2")
                        for k in range(NCK):
                            kk = min(P, HC - k * P)
                            ptr = pst.tile([P, P], bf16, tag="ptr")
                            nc.tensor.transpose(
                                out=ptr[:kk, :],
                                in_=o1[:, bi * HC + k * P:bi * HC + k * P + kk],
                                identity=idn[:])
                            rT = pbb.tile([P, P], bf16, tag="rT")
                            nc.vector.tensor_copy(out=rT[:kk, :], in_=ptr[:kk, :])
                            nc.tensor.matmul(out=ph2[:], lhsT=rT[:kk, :],
                                             rhs=w2s[k][:kk, :],
                                             start=(k == 0), stop=(k == NCK - 1))
                        nc.vector.tensor_copy(out=h2w[:, bi * R2:(bi + 1) * R2],
                                              in_=ph2[:])
                    nc.vector.tensor_copy(
                        out=h2all[:, b0 * R2:(b0 + nblk) * R2],
                        in_=h2w[:, :nblk * R2])
                    nc.sync.dma_start(
                        out=bass.AP(h2loc, b0 * P * R2,
                                    [[R2, P], [P * R2, nblk], [1, R2]]),
                        in_=h2w[:, :nblk * R2].rearrange(
                            "p (g r) -> p g r", g=nblk))

            # ---------------- AllGather (padded per-core rows) ---------------
            nc.gpsimd.collective_compute(
                "AllGather", mybir.AluOpType.bypass,
                replica_groups=[list(range(NC))],
                ins=[h2loc[0:NR2, :]], outs=[h2tabG[:, :]])
            # bounce to a local tensor: gathers from the Shared segment are slow
            nc.sync.dma_start(out=h2tabL[:, :], in_=h2tabG[:, :])

            # ---------------- Phase C: L2 edge pass --------------------------
            with tc.tile_pool(name="pcg", bufs=2) as pcg, \
                 tc.tile_pool(name="pcb", bufs=2) as pcb, \
                 tc.tile_pool(name="psc", bufs=1, space="PSUM") as psc, \
                 tc.tile_pool(name="psk2", bufs=2, space="PSUM") as psk2, \
                 tc.tile_pool(name="psd2", bufs=2, space="PSUM") as psd2:
                ixall2 = pcg.tile([P, Tsum * 8], i16, tag="ixall2", bufs=1)
                nc.sync.dma_start(out=ixall2[:], in_=ihsrc2_d[:, :])
                for sb in sb_meta:
                    base, S = sb["base"], sb["S"]
                    nblk = len(sb["blocks"])
                    b0 = sb["blocks"][0]
                    g2 = pcg.tile([P, S * RL2], f32, tag="g2")
                    # one table: 16 packed nodes per 256B row of h2tabL
                    gather_split(g2, 0, S, RL2,
                                 bass.AP(h2tabL, 0, [[RL2, NC * NG2], [1, RL2]]),
                                 ixall2, ix_base=base)
                    # select each slot's 4 values out of its 16-node row:
                    # expanded mask (k repeated R2x) + contiguous halving adds
                    msel = pcg.tile([P, S * RL2], bf16, tag="msel")
                    nc.vector.tensor_tensor(
                        out=msel[:].rearrange("p (t s) -> p t s", t=S),
                        in0=_sub(svc[:], base, [[1, S], [0, RL2]]),
                        in1=_sub(iotk[:], 0, [[0, S], [1, RL2]]),
                        op=mybir.AluOpType.is_equal)
                    tmps = pcg.tile([P, S * RL2], f32, tag="tmps")
                    nc.vector.tensor_tensor(
                        out=tmps[:], in0=g2[:], in1=msel[:],
                        op=mybir.AluOpType.mult)
                    for half in (32, 16, 8):
                        nc.vector.tensor_tensor(
                            out=_sub(tmps[:], 0, [[RL2, S], [1, half]]),
                            in0=_sub(tmps[:], 0, [[RL2, S], [1, half]]),
                            in1=_sub(tmps[:], half, [[RL2, S], [1, half]]),
                            op=mybir.AluOpType.add)
                    h2m = pcg.tile([P, S * R2], f32, tag="h2m")
                    nc.vector.tensor_tensor(
                        out=h2m[:].rearrange("p (t s) -> p t s", t=S),
                        in0=_sub(tmps[:], 0, [[RL2, S], [1, R2]]),
                        in1=_sub(tmps[:], R2, [[RL2, S], [1, R2]]),
                        op=mybir.AluOpType.add)
                    # a_dst2 window from resident h2all
                    adw2 = pcg.tile([P, 8], bf16, tag="adw2")
                    nc.vector.tensor_copy(
                        out=adw2[:, :nblk],
                        in_=_sub(h2all[:], b0 * R2 + CLS + 1, [[R2, nblk]]))
                    dlT = pcg.tile([1, S * P], bf16, tag="dlT2")
                    nc.sync.dma_start(out=dlT[:],
                                      in_=dlocT_d[0:1, base * P:(base + S) * P])
                    oTa = pcg.tile([P, S * P], bf16, tag="oT2a", bufs=1)
                    oTb = pcg.tile([P, S * P], bf16, tag="oT2b", bufs=1)
                    for st in range(0, S * P, 512):
                        w = min(512, S * P - st)
                        stp = psk2.tile([P, 512], f32, tag="stp2")
                        nc.tensor.matmul(out=stp[:, :w], lhsT=onek[:],
                                         rhs=dlT[0:1, st:st + w],
                                         start=True, stop=True)
                        nc.vector.tensor_tensor(
                            out=oTa[:, st:st + w],
                            in0=iotc2[:, 0:1].to_broadcast([P, w]),
                            in1=stp[:, :w],
                            op=mybir.AluOpType.is_equal)
                        nc.vector.tensor_tensor(
                            out=oTb[:, st:st + w],
                            in0=iotc2[:, 1:2].to_broadcast([P, w]),
                            in1=stp[:, :w],
                            op=mybir.AluOpType.is_equal)
                    pad2 = psd2.tile([P, S], f32, tag="pad2")
                    for td in sb["tiles"]:
                        rel = td["rel"]
                        nmm = len(td["mms"])
                        for mi, (bi, plane) in enumerate(td["mms"]):
                            oT = oTa if plane == 0 else oTb
                            nc.tensor.matmul(
                                out=pad2[:, rel:rel + 1],
                                lhsT=oT[:, rel * P:(rel + 1) * P],
                                rhs=adw2[:, bi:bi + 1],
                                start=(mi == 0), stop=(mi == nmm - 1),
                                skip_group_check=True)
                    ex2 = pcb.tile([P, S], f32, tag="ex2")
                    nc.vector.tensor_tensor(
                        out=ex2[:],
                        in0=_sub(h2m[:], CLS, [[R2, S]]),
                        in1=pad2[:],
                        op=mybir.AluOpType.add)
                    nc.scalar.activation(out=ex2[:], in_=ex2[:],
                                         func=mybir.ActivationFunctionType.Prelu,
                                         alpha=NEG)
                    nc.scalar.activation(out=ex2[:], in_=ex2[:],
                                         func=mybir.ActivationFunctionType.Exp)
                    m2 = pcb.tile([P, S * 3], bf16, tag="m2")
                    nc.vector.tensor_copy(out=_sub(m2[:], CLS, [[3, S]]), in_=ex2[:])
                    nc.vector.tensor_tensor(
                        out=_sub(m2[:], 0, [[3, S], [1, CLS]]),
                        in0=_sub(h2m[:], 0, [[R2, S], [1, CLS]]),
                        in1=_sub(m2[:], CLS, [[3, S], [0, CLS]]),
                        op=mybir.AluOpType.mult)
                    oha = pcb.tile([P, S * P], bf16, tag="oh2a")
                    nc.vector.tensor_tensor(
                        out=oha[:].rearrange("p (t q) -> p t q", t=S),
                        in0=_sub(dlc[:], base, [[1, S], [0, P]]),
                        in1=_sub(iot2[:], 0, [[0, S], [1, P]]),
                        op=mybir.AluOpType.is_equal)
                    ohb = pcb.tile([P, S * P], bf16, tag="oh2b")
                    nc.vector.tensor_tensor(
                        out=ohb[:].rearrange("p (t q) -> p t q", t=S),
                        in0=_sub(dlc[:], base, [[1, S], [0, P]]),
                        in1=_sub(iot2[:], P, [[0, S], [1, P]]),
                        op=mybir.AluOpType.is_equal)
                    # self-loop L2 stats [P, nblk]
                    ex2s = pcb.tile([P, 8], f32, tag="ex2s")
                    nc.vector.tensor_tensor(
                        out=ex2s[:, :nblk],
                        in0=_sub(h2all[:], b0 * R2 + CLS, [[R2, nblk]]),
                        in1=_sub(h2all[:], b0 * R2 + CLS + 1, [[R2, nblk]]),
                        op=mybir.AluOpType.add)
                    nc.scalar.activation(out=ex2s[:, :nblk], in_=ex2s[:, :nblk],
                                         func=mybir.ActivationFunctionType.Prelu,
                                         alpha=NEG)
                    nc.scalar.activation(out=ex2s[:, :nblk], in_=ex2s[:, :nblk],
                                         func=mybir.ActivationFunctionType.Exp)
                    # per-block accumulation into one shared PSUM bank
                    ps24 = psc.tile([P, 4 * P], f32, tag="ps24")
                    for bi, b in enumerate(sb["blocks"]):
                        mms = sb["accum"][b]
                        for mi, (rel, plane) in enumerate(mms):
                            oh = oha if plane == 0 else ohb
                            nc.tensor.matmul(
                                out=ps24[:, bi * P:bi * P + 3],
                                lhsT=oh[:, rel * P:(rel + 1) * P],
                                rhs=m2[:, rel * 3:(rel + 1) * 3],
                                start=(mi == 0), stop=(mi == len(mms) - 1))
                    # batched epilogue
                    tmp2 = pcb.tile([P, 8 * CLS], f32, tag="tmp2")
                    nc.vector.tensor_tensor(
                        out=tmp2[:, :nblk * CLS].rearrange(
                            "p (b c) -> p b c", b=nblk),
                        in0=_sub(h2all[:], b0 * R2, [[R2, nblk], [1, CLS]]),
                        in1=_sub(ex2s[:], 0, [[1, nblk], [0, CLS]]),
                        op=mybir.AluOpType.mult)
                    v0 = pcb.tile([P, 8 * CLS], f32, tag="v0")
                    nc.vector.tensor_tensor(
                        out=v0[:, :nblk * CLS].rearrange(
                            "p (b c) -> p b c", b=nblk),
                        in0=_sub(ps24[:], 0, [[P, nblk], [1, CLS]]),
                        in1=tmp2[:, :nblk * CLS].rearrange(
                            "p (b c) -> p b c", b=nblk),
                        op=mybir.AluOpType.add)
                    den2 = pcb.tile([P, 8], f32, tag="den2")
                    nc.vector.tensor_tensor(
                        out=den2[:, :nblk],
                        in0=_sub(ps24[:], CLS, [[P, nblk]]),
                        in1=ex2s[:, :nblk],
                        op=mybir.AluOpType.add)
                    rd2 = pcb.tile([P, 8], f32, tag="rd2")
                    nc.vector.reciprocal(out=rd2[:, :nblk], in_=den2[:, :nblk])
                    nc.vector.tensor_tensor(
                        out=vall[:, b0 * CLS:(b0 + nblk) * CLS].rearrange(
                            "p (b c) -> p b c", b=nblk),
                        in0=v0[:, :nblk * CLS].rearrange(
                            "p (b c) -> p b c", b=nblk),
                        in1=_sub(rd2[:], 0, [[1, nblk], [0, CLS]]),
                        op=mybir.AluOpType.mult)
                # batched log-softmax: out[:,2b+i] = -ln(1+exp(v_other-v_i))
                nc.vector.tensor_tensor(out=vall[:], in0=vall[:], in1=b2a[:],
                                        op=mybir.AluOpType.add)
                vsw = cp.tile([P, NB * CLS], f32, name="vsw")
                nc.vector.tensor_copy(
                    out=_sub(vsw[:], 0, [[CLS, NB]]),
                    in_=_sub(vall[:], 1, [[CLS, NB]]))
                nc.vector.tensor_copy(
                    out=_sub(vsw[:], 1, [[CLS, NB]]),
                    in_=_sub(vall[:], 0, [[CLS, NB]]))
                nc.vector.tensor_tensor(out=vsw[:], in0=vsw[:], in1=vall[:],
                                        op=mybir.AluOpType.subtract)
                nc.scalar.activation(out=vsw[:], in_=vsw[:],
                                     func=mybir.ActivationFunctionType.Exp)
                nc.vector.tensor_scalar_add(out=vsw[:], in0=vsw[:], scalar1=1.0)
                nc.scalar.activation(out=vsw[:], in_=vsw[:],
                                     func=mybir.ActivationFunctionType.Ln)
                nc.vector.tensor_scalar_mul(out=vsw[:], in0=vsw[:], scalar1=-1.0)
                nfull = NPC // P
                nc.sync.dma_start(
                    out=bass.AP(out_d, 0, [[CLS, P], [P * CLS, nfull], [1, CLS]]),
                    in_=vsw[:, :nfull * CLS].rearrange(
                        "p (g r) -> p g r", g=nfull))
                rows = NPC - nfull * P
                if rows:
                    nc.sync.dma_start(
                        out=out_d[nfull * P:NPC, :],
                        in_=vsw[:rows, nfull * CLS:(nfull + 1) * CLS])
    nc.finalize()
    return nc


def install_ntff_hook(so_path="/opt/axon/libaxon_pjrt.so"):
    import types
    import ctypes
    import contextlib
    import antenv

    if getattr(antenv, "axon_hooks", None) is not None:
        return
    lib = ctypes.CDLL(so_path)
    if not hasattr(lib, "axon_start_nrt_profile"):
        return
    lib.axon_start_nrt_profile.argtypes = [ctypes.POINTER(ctypes.c_int64),
                                           ctypes.c_size_t]
    lib.axon_start_nrt_profile.restype = ctypes.c_int64
    lib.axon_stop_nrt_profile.argtypes = [ctypes.c_char_p]
    lib.axon_stop_nrt_profile.restype = ctypes.c_int64

    @contextlib.contextmanager
    def _hook(output_dir, device_ids):
        import jax
        jax.devices()
        if device_ids:
            ids = (ctypes.c_int64 * len(device_ids))(*device_ids)
            rc = lib.axon_start_nrt_profile(ids, len(device_ids))
        else:
            rc = lib.axon_start_nrt_profile(None, 0)
        if rc != 0:
            raise RuntimeError(f"axon_start_nrt_profile rc={rc}")
        try:
            yield
        finally:
            n = lib.axon_stop_nrt_profile(str(output_dir).encode())
            print(f"ntff profile: {n} file(s) written to {output_dir}")

    mod = types.ModuleType("antenv.axon_hooks")
    _reg = [_hook]
    mod.set_axon_ntff_profile_hook = lambda h: _reg.__setitem__(0, h)
    mod.get_axon_ntff_profile_hook = lambda: _reg[0]
    sys.modules["antenv.axon_hooks"] = mod
    antenv.axon_hooks = mod


def run(inputs, cfg, trace=False, **kwargs):
    if trace:
        install_ntff_hook()
    in_maps, meta = prep(inputs, cfg)
    nc = build(meta)
    res = bass_utils.run_bass_kernel_spmd(
        nc, in_maps, core_ids=list(range(cfg["NC"])), trace=trace, **kwargs)
    out = np.concatenate([res.results[c]["out"] for c in range(cfg["NC"])], axis=0)
    return out, res


_CFG = dict(N=100000, F=165, H=4, C=64, CLS=2, NC=8)


def kernel(**inputs):
    """Full (unsharded) inputs -> full [N, 2] float32 log-softmax output."""
    out, _ = run(inputs, _CFG, trace=False)
    return np.ascontiguousarray(out.astype(np.float32))
